# revision 30
# baseline (speedup 1.0000x reference)
"""Trainium2 Bass kernel for nn_AOGStructure (gnn_message_passing).

Reference computation (per frame f, with NP persons / NO objects, C=256):
    P = pf @ Wp + bp            # persons_red
    A = pf @ Wpr + bpr          # act_persons_red
    O = of @ Wo + bo            # objs_red
    objs_interact[f,i]    = max_j       (P[f,i] @ Wm_obj[:C] + O[f,j] @ Wm_obj[C:] + bm_obj)
    persons_interact[f,i] = max_{j!=i}  (P[f,i] @ Wm_per[:C] + A[f,j] @ Wm_per[C:] + bm_per)
    out = concat([objs_interact, persons_interact], -1)

Since the per-pair message is additive in (i-term, j-term), the max over j
factorizes:  max_j (a_i + b_j) = a_i + max_j b_j.  The [F,NP,NO,C] pair tensor
is never materialized.  For the person block the self-excluded max is computed
from the max and the masked ("second") max.  All biases commute with the max
and are folded into a single per-output-channel bias vector added at the end.

Strategy: data-parallel over frames, 16 frames per core, weights replicated,
no collectives.  A single DMA stream whose transfer order equals PE
consumption order, issued as ~15 large contiguous DMAs (per-DMA issue costs
~0.65us on the SP queue, so small transfers are ruinous):

  phase A   5 chunks of [wpa_k | pf_k]   -> BP/P matmuls   (bf16)
  wm/bias                                -> stage 2 + epilogues
  phase B   4x [wob_g | of_w0_g]         -> OB window-0    (fp8 DoubleRow)
  phase C   4x of_w1_g                   -> OB window-1 (two half-windows)

The whole object path runs in fp8-e4m3: `of` quantized directly, Wob
pre-scaled by 2048 (73% of Wob underflows into e4m3 subnormals unscaled) and
the 1/2048 folded into the per-window max fixup.  Both operands fp8 enables
MatmulPerfMode.DoubleRow: two contraction rows per PE cycle, halving the OB
phase.

The PE HAM clock gate defaults to 1.2GHz and only reaches 2.4GHz after
~3.4us of *continuous* matmul activity; any idle gap restarts the ramp.  So
junk matmuls (into the not-yet-live OB1b PSUM bank) fill every DMA-wait gap:
a burst before phase A and a small burst before each chunk-gated LDWEIGHTS.
Within phase A the BP matmuls run before P per chunk so BP retires first and
the person epilogue (the long DVE chain) starts as early as possible.

Epilogues are split across engines: DVE owns PSUM reads (reduce_max) and the
m2=0 half of the person chain; Pool (gpsimd) gets the m2=1 half plus all
SBUF-only object fixups.  PSUM->SBUF staging copies run on Scalar.  Output is
bf16 in three params (person half / obj window-0 / obj window-1), each DMA'd
on the Scalar queue as soon as its last add lands, so the final transfer is a
64KB sliver instead of the whole output.  PSUM uses exactly 8 banks.
"""

import sys

if "/opt/trn_rl_repo" not in sys.path:
    sys.path.insert(0, "/opt/trn_rl_repo")

import ml_dtypes
import numpy as np

import concourse.bass as bass  # noqa: F401  (import keeps bass registered)
import concourse.tile as tile
from concourse import bacc, mybir
from concourse.bass_utils import run_bass_kernel_spmd

NCORES = 8
F, NP, NO = 128, 16, 48
D, C = 2048, 256
F_LOC = F // NCORES          # 16 frames per core
TP = F_LOC * NP              # 256 person tokens per core
TO = F_LOC * NO              # 768 object tokens per core
KD = D // 128                # 16 contraction chunks of 128
W0 = 384                     # of window 0: frames 0-7
F0 = W0 // NO                # 8 frames in window 0
WOB_SCALE = 2048.0           # keeps fp8 Wob out of the subnormal range
BF16 = ml_dtypes.bfloat16
FP8 = ml_dtypes.float8_e4m3

# k-extents of the five phase-A chunks (first small so the PE starts early)
A_SPLIT = [1, 3, 4, 4, 4]
A_START = [0, 1, 4, 8, 12]
N_WARMUP = 7                 # junk matmuls to ramp the PE clock before data
WAB_SCALE = 128.0            # keeps fp8 Wab out of the subnormal range

_NC_CACHE = None


def _build_nc():
    """Build the single-core SPMD graph (same NEFF on all 8 cores)."""
    nc = bacc.Bacc("TRN2", target_bir_lowering=False, debug=False)
    BF = mybir.dt.bfloat16
    F8 = mybir.dt.float8e4
    F32 = mybir.dt.float32
    DR = mybir.MatmulPerfMode.DoubleRow

    a_d = [
        nc.declare_dram_parameter(f"a{i}", [128, A_SPLIT[i], 512], BF, isOutput=False)
        for i in range(5)
    ]
    # fused BP weight ships as fp8 (x128 to clear the subnormal range) and is
    # consumed DIRECTLY as the fp8 stationary operand against bf16 pf (the PE
    # allows mixed non-fp32 operand dtypes; HW-verified exact) — half the HBM
    # bytes of a bf16 Wab at identical matmul cost.  One chunk per a-chunk,
    # ordered right behind it in the stream.
    wab_d = [
        nc.declare_dram_parameter(f"wab8_{i}", [128, A_SPLIT[i], 256], F8, isOutput=False)
        for i in range(5)
    ]
    # merged fp8 chunks, k-split 8/4/4: per k-row [wob_k (256) | of_w0_k (384)]
    OW_SPLIT = [8, 4, 4]
    OW_START = [0, 8, 12]
    ow_d = [
        nc.declare_dram_parameter(f"ow{h}", [128, OW_SPLIT[h], 640], F8, isOutput=False)
        for h in range(3)
    ]
    # of window 1, k-split 8/4/4 so the final DMA chunk (and the matmul work
    # gated on it) is as small as possible
    OW1_SPLIT = [8, 4, 4]
    OW1_START = [0, 8, 12]
    ow1_d = [
        nc.declare_dram_parameter(f"ow1{h}", [128, OW1_SPLIT[h], W0], F8, isOutput=False)
        for h in range(3)
    ]
    # wm plus the four bias vectors as two extra bf16 columns per row
    wm_d = nc.declare_dram_parameter("wm", [128, 2, 514], BF, isOutput=False)
    # output in three pieces so each can DMA out as soon as it completes:
    # op = person half [c2,c3], ob0 = obj window-0 frames, ob1 = obj window-1
    out_p_d = nc.declare_dram_parameter("out_p", [128, 2, TP], BF, isOutput=True)
    out_b0_d = nc.declare_dram_parameter("out_b0", [128, 2, F0 * NP], BF, isOutput=True)
    out_b1_d = nc.declare_dram_parameter("out_b1", [128, 2, TP - F0 * NP], BF, isOutput=True)

    with tile.TileContext(nc) as tc:
        with (
            tc.tile_pool(name="loads", bufs=1) as loads,
            tc.tile_pool(name="work", bufs=1) as work,
            tc.tile_pool(name="psum", bufs=8, space="PSUM") as psum,
        ):
            # ---- input DMAs on the SP queue, in PE consumption order ----
            a_sb = []
            wab_sb = []
            for i in range(5):
                t = loads.tile([128, A_SPLIT[i], 512], BF, tag=f"a{i}", name=f"a{i}")
                nc.sync.dma_start(t, a_d[i][:, :, :])
                a_sb.append(t)
                t8 = loads.tile([128, A_SPLIT[i], 256], F8, tag=f"wab{i}", name=f"wab{i}")
                nc.sync.dma_start(t8, wab_d[i][:, :, :])
                wab_sb.append(t8)
            wm_sb = loads.tile([128, 2, 514], BF, tag="wm", name="wm")
            nc.sync.dma_start(wm_sb, wm_d[:, :, :])
            ow_sb = [None] * 3
            ow1_sb = [None] * 3
            for h in range(3):
                t = loads.tile([128, OW_SPLIT[h], 640], F8, tag=f"ow{h}", name=f"ow{h}")
                nc.sync.dma_start(t, ow_d[h][:, :, :])
                ow_sb[h] = t
            for h in range(3):
                t = loads.tile([128, OW1_SPLIT[h], W0], F8, tag=f"ow1{h}", name=f"ow1{h}")
                nc.sync.dma_start(t, ow1_d[h][:, :, :])
                ow1_sb[h] = t

            def achunk(k):
                for i in range(4, -1, -1):
                    if k >= A_START[i]:
                        return a_sb[i], k - A_START[i]
                raise AssertionError

            def wpchunk(k, m):  # Wp chunk (feeds P)
                t, kk = achunk(k)
                return t[:, kk, m * 128 : m * 128 + 128]

            def wabchunk(k, m):  # Wab chunk (fp8 stationary, feeds BP)
                t, kk = achunk(k)
                ci = a_sb.index(t)
                return wab_sb[ci][:, kk, m * 128 : m * 128 + 128]

            def pfchunk(k):
                t, kk = achunk(k)
                return t[:, kk, 256:512]

            def wmchunk(kc, sec, m2):  # sec 0 = a_o (Wm1a), 1 = a_p (Wm2a)
                j0 = sec * 256 + m2 * 128
                return wm_sb[:, kc, j0 : j0 + 128]

            def _owc(g, kk):
                k = g * 4 + kk
                c = 0 if k < 8 else (1 if k < 12 else 2)
                return ow_sb[c], k - OW_START[c]

            def wobpair(g, kk, m2):  # [128, 2, 128] fp8 stationary, k-pair
                t, r = _owc(g, kk)
                return t[:, r : r + 2, m2 * 128 : m2 * 128 + 128]

            def ow0pair(g, kk):
                t, r = _owc(g, kk)
                return t[:, r : r + 2, 256:640]

            def ow1pair(g, kk, lo):
                k = g * 4 + kk
                c = 0 if k < 8 else (1 if k < 12 else 2)
                r = k - OW1_START[c]
                return ow1_sb[c][:, r : r + 2, lo : lo + 192]

            # bias views packed into wm: row 0 = object halves, row 1 = person
            def bias_obj(m2):  # [128, 1]
                return wm_sb[:, 0, 512 + m2 : 513 + m2]

            bias_per = wm_sb[:, 1, 512:514]  # [128, 2]

            # ---- PSUM: exactly 8 banks ----
            P_ps = psum.tile([128, 2, TP], F32, tag="ps", name="P_ps")
            BP_ps = psum.tile([128, 2, TP], F32, tag="ps", name="BP_ps")
            AP_ps = psum.tile([128, 2, TP], F32, tag="ps", name="AP_ps")
            AO_ps = psum.tile([128, 2, TP], F32, tag="ps", name="AO_ps")
            OB0 = [psum.tile([128, W0], F32, tag="ps", name=f"OB0_{m2}") for m2 in range(2)]
            OB1a = psum.tile([128, 2, 192], F32, tag="ps", name="OB1a")
            OB1b = psum.tile([128, 2, 192], F32, tag="ps", name="OB1b")

            # ---- PE warmup: the HAM clock gate needs ~3.4us of CONTINUOUS
            #      matmul activity to unthrottle 1.2 -> 2.4GHz, so junk
            #      matmuls fill the pre-data window AND every DMA-wait gap
            #      between phase-A chunks.  Mid-phase junk lands in the OB1b
            #      bank, which has no live accumulation until OB window 1
            #      (whose first real matmul has start=True and clears it). ----
            junk = work.tile([128, 256], BF, tag="junk", name="junk")
            nc.gpsimd.memset(junk, 0)

            def junk_mm(n, tgt=None, ncols=256):
                for _ in range(n):
                    nc.tensor.matmul(
                        tgt if tgt is not None else P_ps[:, 0, :],
                        junk[:, 0:128], junk[:, 0:ncols],
                        start=True, stop=True, skip_group_check=True,
                    )

            junk_mm(N_WARMUP)

            # ---- phase A, paced by a-chunk arrival.  Per chunk, P matmuls
            #      run first (the a-chunk lands first), BP right behind (its
            #      fp8 wab chunk follows in the stream).  Junk bursts bridge
            #      the pre-warm DMA gaps only — once the HAM clock is warm,
            #      short idles cost nothing.  The last chunk runs BP-first,
            #      m-half-major, so the PSUM->SBUF copies (and everything
            #      gated on them) pipeline with the remaining matmuls.
            #      Mid-phase junk lands in OB1b's bank (no live accumulation
            #      there until OB window 1, whose first matmul clears it). ----
            def p_mms(ci, order):
                ks = range(A_START[ci], A_START[ci] + A_SPLIT[ci])
                for m, k in order(ks):
                    nc.tensor.matmul(P_ps[:, m, :], wpchunk(k, m), pfchunk(k),
                                     start=(k == 0 and m == 0),
                                     stop=(k == KD - 1))

            def bp_mms(ci, order):
                ks = range(A_START[ci], A_START[ci] + A_SPLIT[ci])
                for m, k in order(ks):
                    nc.tensor.matmul(BP_ps[:, m, :], wabchunk(k, m), pfchunk(k),
                                     start=(k == 0 and m == 0),
                                     stop=(k == KD - 1))

            k_major = lambda ks: [(m, k) for k in ks for m in range(2)]
            m_major = lambda ks: [(m, k) for m in range(2) for k in ks]
            for ci in range(4):
                p_mms(ci, k_major)
                bp_mms(ci, k_major)
                if ci == 0:
                    junk_mm(3, tgt=OB1b[:, 0, :], ncols=192)
                elif ci == 1:
                    junk_mm(2, tgt=OB1b[:, 0, :], ncols=192)
            bp_mms(4, m_major)
            p_mms(4, m_major)

            # BP/P PSUM -> SBUF per m-half on Scalar, in retirement order, so
            # the person epilogue and stage 2 start as early as possible.  The
            # BP copy folds the 1/WAB_SCALE descale.
            CP = mybir.ActivationFunctionType.Copy
            BPc = work.tile([128, 2, TP], BF, tag="BPc", name="BPc")
            PT = work.tile([128, 2, TP], BF, tag="PTsb", name="PTsb")
            for m in range(2):
                nc.scalar.activation(BPc[:, m, :], BP_ps[:, m, :], CP,
                                     scale=1.0 / WAB_SCALE)
            for m in range(2):
                nc.scalar.copy(PT[:, m, :], P_ps[:, m, :])

            # ---- stage 2: a_p (AP) and a_o (AO) from PT, kc-outer so the
            #      kc=0 matmuls only wait on PT's first half ----
            for kc in range(2):
                for m2 in range(2):
                    st, sp = (kc == 0 and m2 == 0), (kc == 1)
                    nc.tensor.matmul(AP_ps[:, m2, :], wmchunk(kc, 1, m2), PT[:, kc, :], start=st, stop=sp)
                    nc.tensor.matmul(AO_ps[:, m2, :], wmchunk(kc, 0, m2), PT[:, kc, :], start=st, stop=sp)

            # ---- OB window 0: fp8 DoubleRow, two k-planes per matmul ----
            for g in range(4):
                for kk in (0, 2):
                    k = g * 4 + kk
                    for m2 in range(2):
                        nc.tensor.matmul(
                            OB0[m2], wobpair(g, kk, m2),
                            ow0pair(g, kk),
                            start=(k == 0), stop=(k == KD - 2),
                            perf_mode=DR,
                        )

            # ---- remaining PSUM -> SBUF copies (Scalar) ----
            APc = work.tile([128, 2, TP], BF, tag="APc", name="APc")
            nc.scalar.copy(APc, AP_ps)
            AOc = work.tile([128, 2, TP], BF, tag="AOc", name="AOc")
            nc.scalar.copy(AOc, AO_ps)

            # ---- persons epilogue (self-excluded max), bf16 on DVE
            # (free-axis reductions are DVE-only on TRN2; bf16 doubles DVE
            # throughput) ----
            SH3, SH4 = (128, 2, F_LOC), (128, 2, F_LOC, NP)
            out_p = work.tile([128, 2, TP], BF, tag="out_p", name="out_p")
            out_b0 = work.tile([128, 2, F0 * NP], BF, tag="out_b0", name="out_b0")
            out_b1 = work.tile([128, 2, TP - F0 * NP], BF, tag="out_b1", name="out_b1")
            V = nc.vector
            G = nc.gpsimd
            bp4 = BPc.rearrange("p c (f i) -> p c f i", i=NP)
            m1 = work.tile(list(SH3), BF, tag="m1", name="m1")
            V.reduce_max(m1, bp4, axis=mybir.AxisListType.X)
            eq = work.tile(list(SH4), BF, tag="eq", name="eq")
            V.tensor_tensor(eq, bp4, m1[:, :, :, None].to_broadcast(SH4),
                            mybir.AluOpType.is_equal)
            msk = work.tile(list(SH4), BF, tag="msk", name="msk")
            V.scalar_tensor_tensor(msk, eq, -1e30, bp4,
                                   mybir.AluOpType.mult, mybir.AluOpType.add)
            m2v = work.tile(list(SH3), BF, tag="m2v", name="m2v")
            V.reduce_max(m2v, msk, axis=mybir.AxisListType.X)
            dd = work.tile(list(SH3), BF, tag="dd", name="dd")
            V.tensor_tensor(dd, m2v, m1, mybir.AluOpType.subtract)
            m1pb = work.tile(list(SH3), BF, tag="m1pb", name="m1pb")
            V.tensor_tensor(m1pb, m1, bias_per[:, :, None].to_broadcast(SH3),
                            mybir.AluOpType.add)
            mex = work.tile(list(SH4), BF, tag="mex", name="mex")
            V.tensor_tensor(mex, eq, dd[:, :, :, None].to_broadcast(SH4),
                            mybir.AluOpType.mult)
            V.tensor_tensor(mex, mex, m1pb[:, :, :, None].to_broadcast(SH4),
                            mybir.AluOpType.add)
            # final add on Pool: frees DVE for the object reduces
            G.tensor_tensor(
                out_p.rearrange("p c (f i) -> p c f i", i=NP),
                APc.rearrange("p c (f i) -> p c f i", i=NP),
                mex, mybir.AluOpType.add,
            )
            nc.scalar.dma_start(out_p_d[:, :, :], out_p)

            # ---- OB window 1: k-pair outer, both half-windows inner, so each
            #      incoming ow1 chunk is fully consumed before the next is
            #      needed and the post-last-chunk matmul tail is 8 MMs.  The
            #      a-half's stop lands before the b-half's, so its reduce
            #      overlaps the b-half's final matmuls. ----
            for g in range(4):
                for kk in (0, 2):
                    k = g * 4 + kk
                    for OB1, lo in ((OB1a, 0), (OB1b, 192)):
                        for m2 in range(2):
                            nc.tensor.matmul(
                                OB1[:, m2, :], wobpair(g, kk, m2),
                                ow1pair(g, kk, lo),
                                start=(k == 0 and m2 == 0),
                                stop=(k == KD - 2),
                                perf_mode=DR,
                            )

            # ---- object epilogues ----
            # Reduces straight from PSUM on DVE (the only engine that can),
            # descale on the tiny maxo tile, then one fused
            # scalar_tensor_tensor per (window, channel-half) on DVE — Pool
            # turned out to be ~2x slower per element with ~0.5us fixed cost,
            # so only the person final-add (fully hidden) lives there.
            maxo = work.tile([128, 2, 2, F0], F32, tag="maxo", name="maxo")

            def obj_add(w, m2, hslc, t0, nfr, dst, d0):
                V.scalar_tensor_tensor(
                    dst[:, m2, d0 : d0 + nfr * NP].rearrange(
                        "p (f i) -> p f i", i=NP
                    ),
                    maxo[:, w, m2, hslc, None].to_broadcast((128, nfr, NP)),
                    bias_obj(m2),
                    AOc[:, m2, t0 : t0 + nfr * NP].rearrange("p (f i) -> p f i", i=NP),
                    mybir.AluOpType.add,
                    mybir.AluOpType.add,
                )

            # window 0 (frames 0-7), one reduce per OB0 psum bank
            for m2 in range(2):
                V.reduce_max(
                    maxo[:, 0, m2, :],
                    OB0[m2].rearrange("p (f o) -> p f o", o=NO),
                    axis=mybir.AxisListType.X,
                )
            V.tensor_scalar_mul(maxo[:, 0, :, :], maxo[:, 0, :, :], 1.0 / WOB_SCALE)
            for m2 in range(2):
                obj_add(0, m2, slice(0, F0), 0, F0, out_b0, 0)
            nc.scalar.dma_start(out_b0_d[:, :, :], out_b0)

            # window 1, half A (frames 8-11) then half B (12-15)
            for h, OB1 in ((0, OB1a), (1, OB1b)):
                hs = slice(4 * h, 4 * h + 4)
                V.reduce_max(
                    maxo[:, 1, :, hs],
                    OB1.rearrange("p c (f o) -> p c f o", o=NO),
                    axis=mybir.AxisListType.X,
                )
                V.tensor_scalar_mul(maxo[:, 1, :, hs], maxo[:, 1, :, hs], 1.0 / WOB_SCALE)
                for m2 in range(2):
                    obj_add(1, m2, hs, 128 + 64 * h, 4, out_b1, 64 * h)
            nc.scalar.dma_start(out_b1_d[:, :, :], out_b1)

    nc.compile()
    return nc


def _get_nc():
    global _NC_CACHE
    if _NC_CACHE is None:
        _NC_CACHE = _build_nc()
    return _NC_CACHE


def _marshal(pf, of, Wp, bp, Wpr, bpr, Wo, bo, Wm_obj, bm_obj, Wm_per, bm_per):
    """Pack full f32 inputs into per-core DRAM parameter layouts."""
    pf_bf = pf.astype(BF16)
    of_q = of.astype(FP8)

    Wab = Wpr @ Wm_per[C:]                                               # [D, C] fused BP weight
    Wob = Wo @ Wm_obj[C:]                                                # [D, C] fused OB weight
    wp_packed = Wp.astype(BF16).reshape(KD, 128, 256).transpose(1, 0, 2)  # [128, KD, 256]
    wab_packed = (Wab * WAB_SCALE).astype(FP8).reshape(KD, 128, 256).transpose(1, 0, 2)
    wob_packed = (Wob * WOB_SCALE).astype(FP8).reshape(KD, 128, 256).transpose(1, 0, 2)
    wmcat = np.concatenate([Wm_obj[:C], Wm_per[:C]], axis=1).astype(BF16)  # [C, 512]
    wm_packed = wmcat.reshape(2, 128, 512).transpose(1, 0, 2)            # [128, 2, 512]

    bias_obj = bm_obj + bp @ Wm_obj[:C] + bo @ Wm_obj[C:]
    bias_per = bm_per + bp @ Wm_per[:C] + bpr @ Wm_per[C:]
    # bias rides in wm as two extra bf16 columns: row 0 obj halves, row 1 per
    bias4 = np.stack(
        [bias_obj[0:128], bias_obj[128:256], bias_per[0:128], bias_per[128:256]],
        axis=1,
    ).astype(BF16)                                                       # [128, 4]
    wmb = np.concatenate([wm_packed, bias4.reshape(128, 2, 2)], axis=2)  # [128, 2, 514]
    wmb = np.ascontiguousarray(wmb)

    in_maps = []
    for c in range(NCORES):
        pfc = pf_bf[c * TP : (c + 1) * TP]                                # [TP, D]
        ofc = of_q[c * TO : (c + 1) * TO]                                 # [TO, D]
        pf_packed = pfc.reshape(TP, KD, 128).transpose(2, 1, 0)           # [128, KD, TP]
        a_full = np.concatenate([wp_packed, pf_packed], axis=2)           # [128, KD, 512]
        of_packed = ofc.reshape(TO, KD, 128).transpose(2, 1, 0)           # [128, KD, TO]
        owcat = np.concatenate(
            [wob_packed, of_packed[:, :, 0:W0]], axis=2
        )                                                                 # [128, KD, 640]
        m = {"wm": wmb}
        for i in range(5):
            m[f"a{i}"] = np.ascontiguousarray(
                a_full[:, A_START[i] : A_START[i] + A_SPLIT[i], :]
            )
            m[f"wab8_{i}"] = np.ascontiguousarray(
                wab_packed[:, A_START[i] : A_START[i] + A_SPLIT[i], :]
            )
        for h, (k0, kn) in enumerate(((0, 8), (8, 4), (12, 4))):
            m[f"ow{h}"] = np.ascontiguousarray(owcat[:, k0 : k0 + kn, :])
            m[f"ow1{h}"] = np.ascontiguousarray(
                of_packed[:, k0 : k0 + kn, W0:TO]
            )
        in_maps.append(m)
    return in_maps


def _unmarshal(results):
    """Per-core 3-piece bf16 output -> [F*NP, 2C, 1,1,1] f32."""
    blocks = []
    for c in range(NCORES):
        r = results[c]
        obj = np.concatenate(
            [np.asarray(r["out_b0"]), np.asarray(r["out_b1"])], axis=2
        ).astype(np.float32)                                              # [128, 2, TP]
        per = np.asarray(r["out_p"]).astype(np.float32)                   # [128, 2, TP]
        arr = np.concatenate([obj, per], axis=1)                          # [128, 4, TP]
        out_t = arr.transpose(1, 0, 2).reshape(2 * C, TP)                 # [512, TP]
        blocks.append(out_t.T)                                            # [TP, 512]
    full = np.concatenate(blocks, axis=0).astype(np.float32)              # [F*NP, 2C]
    return full[:, :, None, None, None]


def kernel(
    person_feature,
    obj_feature,
    Wp,
    bp,
    Wpr,
    bpr,
    Wo,
    bo,
    Wm_obj,
    bm_obj,
    Wm_per,
    bm_per,
    f_num,
    np_pf,
    no_pf,
):
    assert int(f_num) == F and int(np_pf) == NP and int(no_pf) == NO
    pf = np.asarray(person_feature, dtype=np.float32)[:, :, 0, 0, 0]
    of = np.asarray(obj_feature, dtype=np.float32)[:, :, 0, 0, 0]
    args = [
        np.asarray(a, dtype=np.float32)
        for a in (Wp, bp, Wpr, bpr, Wo, bo, Wm_obj, bm_obj, Wm_per, bm_per)
    ]
    in_maps = _marshal(pf, of, *args)
    nc = _get_nc()
    res = run_bass_kernel_spmd(nc, in_maps, core_ids=list(range(NCORES)))
    return _unmarshal(res.results)


if __name__ == "__main__":
    # smoke test with random data against a numpy re-derivation
    rng = np.random.default_rng(0)
    pf = rng.standard_normal((F * NP, D, 1, 1, 1), dtype=np.float32)
    of = rng.standard_normal((F * NO, D, 1, 1, 1), dtype=np.float32)
    mk = lambda *s: (rng.standard_normal(s, dtype=np.float32) * 0.01)
    inputs = dict(
        person_feature=pf,
        obj_feature=of,
        Wp=mk(D, C),
        bp=np.zeros(C, np.float32),
        Wpr=mk(D, C),
        bpr=np.zeros(C, np.float32),
        Wo=mk(D, C),
        bo=np.zeros(C, np.float32),
        Wm_obj=rng.standard_normal((2 * C, C), dtype=np.float32) / np.sqrt(2 * C),
        bm_obj=np.zeros(C, np.float32),
        Wm_per=rng.standard_normal((2 * C, C), dtype=np.float32) / np.sqrt(2 * C),
        bm_per=np.zeros(C, np.float32),
        f_num=F,
        np_pf=NP,
        no_pf=NO,
    )
    out = kernel(**inputs)
    print("kernel output shape:", out.shape)


# revision 35
# speedup vs baseline: 1.0636x; 1.0636x over previous
"""Trainium2 Bass kernel for nn_AOGStructure (gnn_message_passing).

Reference computation (per frame f, with NP persons / NO objects, C=256):
    P = pf @ Wp + bp            # persons_red
    A = pf @ Wpr + bpr          # act_persons_red
    O = of @ Wo + bo            # objs_red
    objs_interact[f,i]    = max_j       (P[f,i] @ Wm_obj[:C] + O[f,j] @ Wm_obj[C:] + bm_obj)
    persons_interact[f,i] = max_{j!=i}  (P[f,i] @ Wm_per[:C] + A[f,j] @ Wm_per[C:] + bm_per)
    out = concat([objs_interact, persons_interact], -1)

Since the per-pair message is additive in (i-term, j-term), the max over j
factorizes:  max_j (a_i + b_j) = a_i + max_j b_j.  The [F,NP,NO,C] pair tensor
is never materialized.  For the person block the self-excluded max is computed
from the max and the masked ("second") max.  All biases commute with the max
and are folded into a single per-output-channel bias vector added at the end.

Strategy: data-parallel over frames, 16 frames per core, weights replicated,
no collectives.  A single DMA stream whose transfer order equals PE
consumption order, issued as ~15 large contiguous DMAs (per-DMA issue costs
~0.65us on the SP queue, so small transfers are ruinous):

  phase A   5 chunks of [wpa_k | pf_k]   -> BP/P matmuls   (bf16)
  wm/bias                                -> stage 2 + epilogues
  phase B   4x [wob_g | of_w0_g]         -> OB window-0    (fp8 DoubleRow)
  phase C   4x of_w1_g                   -> OB window-1 (two half-windows)

The whole object path runs in fp8-e4m3: `of` quantized directly, Wob
pre-scaled by 2048 (73% of Wob underflows into e4m3 subnormals unscaled) and
the 1/2048 folded into the per-window max fixup.  Both operands fp8 enables
MatmulPerfMode.DoubleRow: two contraction rows per PE cycle, halving the OB
phase.

The PE HAM clock gate defaults to 1.2GHz and only reaches 2.4GHz after
~3.4us of *continuous* matmul activity; any idle gap restarts the ramp.  So
junk matmuls (into the not-yet-live OB1b PSUM bank) fill every DMA-wait gap:
a burst before phase A and a small burst before each chunk-gated LDWEIGHTS.
Within phase A the BP matmuls run before P per chunk so BP retires first and
the person epilogue (the long DVE chain) starts as early as possible.

Epilogues are split across engines: DVE owns PSUM reads (reduce_max) and the
m2=0 half of the person chain; Pool (gpsimd) gets the m2=1 half plus all
SBUF-only object fixups.  PSUM->SBUF staging copies run on Scalar.  Output is
bf16 in three params (person half / obj window-0 / obj window-1), each DMA'd
on the Scalar queue as soon as its last add lands, so the final transfer is a
64KB sliver instead of the whole output.  PSUM uses exactly 8 banks.
"""

import sys

if "/opt/trn_rl_repo" not in sys.path:
    sys.path.insert(0, "/opt/trn_rl_repo")

import ml_dtypes
import numpy as np

import concourse.bass as bass  # noqa: F401  (import keeps bass registered)
import concourse.tile as tile
from concourse import bacc, mybir
from concourse.bass_utils import run_bass_kernel_spmd

NCORES = 8
F, NP, NO = 128, 16, 48
D, C = 2048, 256
F_LOC = F // NCORES          # 16 frames per core
TP = F_LOC * NP              # 256 person tokens per core
TO = F_LOC * NO              # 768 object tokens per core
KD = D // 128                # 16 contraction chunks of 128
W0 = 384                     # of window 0: frames 0-7
F0 = W0 // NO                # 8 frames in window 0
WOB_SCALE = 2048.0           # keeps fp8 Wob out of the subnormal range
BF16 = ml_dtypes.bfloat16
FP8 = ml_dtypes.float8_e4m3

# k-extents of the five phase-A chunks (first small so the PE starts early)
A_SPLIT = [1, 3, 4, 4, 4]
A_START = [0, 1, 4, 8, 12]
N_WARMUP = 12                # junk matmuls to ramp the PE clock before data
WAB_SCALE = 128.0            # keeps fp8 Wab out of the subnormal range

_NC_CACHE = None


def _build_nc():
    """Build the single-core SPMD graph (same NEFF on all 8 cores)."""
    nc = bacc.Bacc("TRN2", target_bir_lowering=False, debug=False)
    BF = mybir.dt.bfloat16
    F8 = mybir.dt.float8e4
    F32 = mybir.dt.float32
    DR = mybir.MatmulPerfMode.DoubleRow

    a_d = [
        nc.declare_dram_parameter(f"a{i}", [128, A_SPLIT[i], 512], BF, isOutput=False)
        for i in range(5)
    ]
    # fused BP weight ships as fp8 (x128 to clear the subnormal range) and is
    # consumed DIRECTLY as the fp8 stationary operand against bf16 pf (the PE
    # allows mixed non-fp32 operand dtypes; HW-verified exact) — half the HBM
    # bytes of a bf16 Wab at identical matmul cost.  One chunk per a-chunk,
    # ordered right behind it in the stream.
    wab_d = nc.declare_dram_parameter("wab8", [128, KD, 256], F8, isOutput=False)
    # merged fp8 chunks, k-split 8/4/4: per k-row [wob_k (256) | of_w0_k (384)]
    OW_SPLIT = [8, 4, 4]
    OW_START = [0, 8, 12]
    ow_d = [
        nc.declare_dram_parameter(f"ow{h}", [128, OW_SPLIT[h], 640], F8, isOutput=False)
        for h in range(3)
    ]
    # of window 1, k-split 8/4/4 so the final DMA chunk (and the matmul work
    # gated on it) is as small as possible
    OW1_SPLIT = [8, 4, 4]
    OW1_START = [0, 8, 12]
    ow1_d = [
        nc.declare_dram_parameter(f"ow1{h}", [128, OW1_SPLIT[h], W0], F8, isOutput=False)
        for h in range(3)
    ]
    # wm plus the four bias vectors as two extra bf16 columns per row
    wm_d = nc.declare_dram_parameter("wm", [128, 2, 514], BF, isOutput=False)
    # output in three pieces so each can DMA out as soon as it completes:
    # op = person half [c2,c3], ob0 = obj window-0 frames, ob1 = obj window-1
    out_p_d = nc.declare_dram_parameter("out_p", [128, 2, TP], BF, isOutput=True)
    out_b0_d = nc.declare_dram_parameter("out_b0", [128, 2, F0 * NP], BF, isOutput=True)
    out_b1_d = nc.declare_dram_parameter("out_b1", [128, 2, TP - F0 * NP], BF, isOutput=True)

    with tile.TileContext(nc) as tc:
        with (
            tc.tile_pool(name="loads", bufs=1) as loads,
            tc.tile_pool(name="work", bufs=1) as work,
            tc.tile_pool(name="psum", bufs=8, space="PSUM") as psum,
        ):
            # ---- input DMAs on the SP queue, in PE consumption order.  The
            #      fp8 wab rides as ONE transfer after a1 — many small
            #      transfers (sub-512B per partition line) starve the stream
            #      on issue cost and RMW penalties. ----
            a_sb = []
            wab_sb = None
            for i in range(5):
                t = loads.tile([128, A_SPLIT[i], 512], BF, tag=f"a{i}", name=f"a{i}")
                nc.sync.dma_start(t, a_d[i][:, :, :])
                a_sb.append(t)
                if i == 1:
                    wab_sb = loads.tile([128, KD, 256], F8, tag="wab", name="wab")
                    nc.sync.dma_start(wab_sb, wab_d[:, :, :])
            wm_sb = loads.tile([128, 2, 514], BF, tag="wm", name="wm")
            nc.sync.dma_start(wm_sb, wm_d[:, :, :])
            ow_sb = [None] * 3
            ow1_sb = [None] * 3
            for h in range(3):
                t = loads.tile([128, OW_SPLIT[h], 640], F8, tag=f"ow{h}", name=f"ow{h}")
                nc.sync.dma_start(t, ow_d[h][:, :, :])
                ow_sb[h] = t
            for h in range(3):
                t = loads.tile([128, OW1_SPLIT[h], W0], F8, tag=f"ow1{h}", name=f"ow1{h}")
                nc.sync.dma_start(t, ow1_d[h][:, :, :])
                ow1_sb[h] = t

            def achunk(k):
                for i in range(4, -1, -1):
                    if k >= A_START[i]:
                        return a_sb[i], k - A_START[i]
                raise AssertionError

            def wpchunk(k, m):  # Wp chunk (feeds P)
                t, kk = achunk(k)
                return t[:, kk, m * 128 : m * 128 + 128]

            def wabchunk(k, m):  # Wab chunk (fp8 stationary, feeds BP)
                return wab_sb[:, k, m * 128 : m * 128 + 128]

            def pfchunk(k):
                t, kk = achunk(k)
                return t[:, kk, 256:512]

            def wmchunk(kc, sec, m2):  # sec 0 = a_o (Wm1a), 1 = a_p (Wm2a)
                j0 = sec * 256 + m2 * 128
                return wm_sb[:, kc, j0 : j0 + 128]

            def _owc(g, kk):
                k = g * 4 + kk
                c = 0 if k < 8 else (1 if k < 12 else 2)
                return ow_sb[c], k - OW_START[c]

            def wobpair(g, kk, m2):  # [128, 2, 128] fp8 stationary, k-pair
                t, r = _owc(g, kk)
                return t[:, r : r + 2, m2 * 128 : m2 * 128 + 128]

            def ow0pair(g, kk):
                t, r = _owc(g, kk)
                return t[:, r : r + 2, 256:640]

            def ow1pair(g, kk, lo):
                k = g * 4 + kk
                c = 0 if k < 8 else (1 if k < 12 else 2)
                r = k - OW1_START[c]
                return ow1_sb[c][:, r : r + 2, lo : lo + 192]

            # bias views packed into wm: row 0 = object halves, row 1 = person
            def bias_obj(m2):  # [128, 1]
                return wm_sb[:, 0, 512 + m2 : 513 + m2]

            bias_per = wm_sb[:, 1, 512:514]  # [128, 2]

            # ---- PSUM: exactly 8 banks ----
            P_ps = psum.tile([128, 2, TP], F32, tag="ps", name="P_ps")
            BP_ps = psum.tile([128, 2, TP], F32, tag="ps", name="BP_ps")
            AP_ps = psum.tile([128, 2, TP], F32, tag="ps", name="AP_ps")
            AO_ps = psum.tile([128, 2, TP], F32, tag="ps", name="AO_ps")
            OB0 = [psum.tile([128, W0], F32, tag="ps", name=f"OB0_{m2}") for m2 in range(2)]
            OB1a = psum.tile([128, 2, 192], F32, tag="ps", name="OB1a")
            OB1b = psum.tile([128, 2, 192], F32, tag="ps", name="OB1b")

            # ---- PE warmup: the HAM clock gate needs ~3.4us of CONTINUOUS
            #      matmul activity to unthrottle 1.2 -> 2.4GHz, so junk
            #      matmuls fill the pre-data window AND every DMA-wait gap
            #      between phase-A chunks.  Mid-phase junk lands in the OB1b
            #      bank, which has no live accumulation until OB window 1
            #      (whose first real matmul has start=True and clears it). ----
            junk = work.tile([128, 256], BF, tag="junk", name="junk")
            nc.gpsimd.memset(junk, 0)

            def junk_mm(n, tgt=None, ncols=256):
                for _ in range(n):
                    nc.tensor.matmul(
                        tgt if tgt is not None else P_ps[:, 0, :],
                        junk[:, 0:128], junk[:, 0:ncols],
                        start=True, stop=True, skip_group_check=True,
                    )

            junk_mm(N_WARMUP)

            # ---- phase A, paced by a-chunk arrival.  Per chunk, P matmuls
            #      run first (the a-chunk lands first), BP right behind (its
            #      fp8 wab chunk follows in the stream).  Junk bursts bridge
            #      the pre-warm DMA gaps only — once the HAM clock is warm,
            #      short idles cost nothing.  The last chunk runs BP-first,
            #      m-half-major, so the PSUM->SBUF copies (and everything
            #      gated on them) pipeline with the remaining matmuls.
            #      Mid-phase junk lands in OB1b's bank (no live accumulation
            #      there until OB window 1, whose first matmul clears it). ----
            def p_mms(ci, order):
                ks = range(A_START[ci], A_START[ci] + A_SPLIT[ci])
                for m, k in order(ks):
                    nc.tensor.matmul(P_ps[:, m, :], wpchunk(k, m), pfchunk(k),
                                     start=(k == 0 and m == 0),
                                     stop=(k == KD - 1))

            def bp_mms(ci, order):
                ks = range(A_START[ci], A_START[ci] + A_SPLIT[ci])
                for m, k in order(ks):
                    nc.tensor.matmul(BP_ps[:, m, :], wabchunk(k, m), pfchunk(k),
                                     start=(k == 0 and m == 0),
                                     stop=(k == KD - 1))

            k_major = lambda ks: [(m, k) for k in ks for m in range(2)]
            m_major = lambda ks: [(m, k) for m in range(2) for k in ks]
            # chunks 0-1: P only (wab is still in flight), junk keeps the
            # HAM ramp gap-free; BP catches up once wab lands
            p_mms(0, k_major)
            junk_mm(4, tgt=OB1b[:, 0, :], ncols=192)
            p_mms(1, k_major)
            junk_mm(3, tgt=OB1b[:, 0, :], ncols=192)
            bp_mms(0, k_major)
            bp_mms(1, k_major)
            for ci in (2, 3):
                p_mms(ci, k_major)
                bp_mms(ci, k_major)
            bp_mms(4, m_major)
            p_mms(4, m_major)

            # BP/P PSUM -> SBUF per m-half on Scalar, in retirement order, so
            # the person epilogue and stage 2 start as early as possible.  The
            # BP copy folds the 1/WAB_SCALE descale.
            CP = mybir.ActivationFunctionType.Copy
            BPc = work.tile([128, 2, TP], BF, tag="BPc", name="BPc")
            PT = work.tile([128, 2, TP], BF, tag="PTsb", name="PTsb")
            for m in range(2):
                nc.scalar.activation(BPc[:, m, :], BP_ps[:, m, :], CP,
                                     scale=1.0 / WAB_SCALE)
            for m in range(2):
                nc.scalar.copy(PT[:, m, :], P_ps[:, m, :])

            # ---- stage 2: a_p (AP) and a_o (AO) from PT, kc-outer so the
            #      kc=0 matmuls only wait on PT's first half ----
            for kc in range(2):
                for m2 in range(2):
                    st, sp = (kc == 0 and m2 == 0), (kc == 1)
                    nc.tensor.matmul(AP_ps[:, m2, :], wmchunk(kc, 1, m2), PT[:, kc, :], start=st, stop=sp)
                    nc.tensor.matmul(AO_ps[:, m2, :], wmchunk(kc, 0, m2), PT[:, kc, :], start=st, stop=sp)

            # ---- OB window 0: fp8 DoubleRow, two k-planes per matmul ----
            for g in range(4):
                for kk in (0, 2):
                    k = g * 4 + kk
                    for m2 in range(2):
                        nc.tensor.matmul(
                            OB0[m2], wobpair(g, kk, m2),
                            ow0pair(g, kk),
                            start=(k == 0), stop=(k == KD - 2),
                            perf_mode=DR,
                        )

            # ---- remaining PSUM -> SBUF copies (Scalar) ----
            APc = work.tile([128, 2, TP], BF, tag="APc", name="APc")
            nc.scalar.copy(APc, AP_ps)
            AOc = work.tile([128, 2, TP], BF, tag="AOc", name="AOc")
            nc.scalar.copy(AOc, AO_ps)

            # ---- persons epilogue (self-excluded max), bf16 on DVE
            # (free-axis reductions are DVE-only on TRN2; bf16 doubles DVE
            # throughput) ----
            SH3, SH4 = (128, 2, F_LOC), (128, 2, F_LOC, NP)
            out_p = work.tile([128, 2, TP], BF, tag="out_p", name="out_p")
            out_b0 = work.tile([128, 2, F0 * NP], BF, tag="out_b0", name="out_b0")
            out_b1 = work.tile([128, 2, TP - F0 * NP], BF, tag="out_b1", name="out_b1")
            V = nc.vector
            G = nc.gpsimd
            bp4 = BPc.rearrange("p c (f i) -> p c f i", i=NP)
            m1 = work.tile(list(SH3), BF, tag="m1", name="m1")
            V.reduce_max(m1, bp4, axis=mybir.AxisListType.X)
            eq = work.tile(list(SH4), BF, tag="eq", name="eq")
            V.tensor_tensor(eq, bp4, m1[:, :, :, None].to_broadcast(SH4),
                            mybir.AluOpType.is_equal)
            msk = work.tile(list(SH4), BF, tag="msk", name="msk")
            V.scalar_tensor_tensor(msk, eq, -1e30, bp4,
                                   mybir.AluOpType.mult, mybir.AluOpType.add)
            m2v = work.tile(list(SH3), BF, tag="m2v", name="m2v")
            V.reduce_max(m2v, msk, axis=mybir.AxisListType.X)
            dd = work.tile(list(SH3), BF, tag="dd", name="dd")
            V.tensor_tensor(dd, m2v, m1, mybir.AluOpType.subtract)
            m1pb = work.tile(list(SH3), BF, tag="m1pb", name="m1pb")
            V.tensor_tensor(m1pb, m1, bias_per[:, :, None].to_broadcast(SH3),
                            mybir.AluOpType.add)
            mex = work.tile(list(SH4), BF, tag="mex", name="mex")
            V.tensor_tensor(mex, eq, dd[:, :, :, None].to_broadcast(SH4),
                            mybir.AluOpType.mult)
            V.tensor_tensor(mex, mex, m1pb[:, :, :, None].to_broadcast(SH4),
                            mybir.AluOpType.add)
            # final add on Pool: frees DVE for the object reduces
            G.tensor_tensor(
                out_p.rearrange("p c (f i) -> p c f i", i=NP),
                APc.rearrange("p c (f i) -> p c f i", i=NP),
                mex, mybir.AluOpType.add,
            )
            nc.scalar.dma_start(out_p_d[:, :, :], out_p)

            # ---- OB window 1: k-pair outer, both half-windows inner, so each
            #      incoming ow1 chunk is fully consumed before the next is
            #      needed and the post-last-chunk matmul tail is 8 MMs.  The
            #      a-half's stop lands before the b-half's, so its reduce
            #      overlaps the b-half's final matmuls. ----
            for g in range(4):
                for kk in (0, 2):
                    k = g * 4 + kk
                    for OB1, lo in ((OB1a, 0), (OB1b, 192)):
                        for m2 in range(2):
                            nc.tensor.matmul(
                                OB1[:, m2, :], wobpair(g, kk, m2),
                                ow1pair(g, kk, lo),
                                start=(k == 0 and m2 == 0),
                                stop=(k == KD - 2),
                                perf_mode=DR,
                            )

            # ---- object epilogues ----
            # Reduces straight from PSUM on DVE (the only engine that can),
            # descale on the tiny maxo tile, then one fused
            # scalar_tensor_tensor per (window, channel-half) on DVE — Pool
            # turned out to be ~2x slower per element with ~0.5us fixed cost,
            # so only the person final-add (fully hidden) lives there.
            maxo = work.tile([128, 2, 2, F0], F32, tag="maxo", name="maxo")

            def obj_add(w, m2, hslc, t0, nfr, dst, d0):
                V.scalar_tensor_tensor(
                    dst[:, m2, d0 : d0 + nfr * NP].rearrange(
                        "p (f i) -> p f i", i=NP
                    ),
                    maxo[:, w, m2, hslc, None].to_broadcast((128, nfr, NP)),
                    bias_obj(m2),
                    AOc[:, m2, t0 : t0 + nfr * NP].rearrange("p (f i) -> p f i", i=NP),
                    mybir.AluOpType.add,
                    mybir.AluOpType.add,
                )

            # window 0 (frames 0-7), one reduce per OB0 psum bank
            for m2 in range(2):
                V.reduce_max(
                    maxo[:, 0, m2, :],
                    OB0[m2].rearrange("p (f o) -> p f o", o=NO),
                    axis=mybir.AxisListType.X,
                )
            V.tensor_scalar_mul(maxo[:, 0, :, :], maxo[:, 0, :, :], 1.0 / WOB_SCALE)
            for m2 in range(2):
                obj_add(0, m2, slice(0, F0), 0, F0, out_b0, 0)
            nc.scalar.dma_start(out_b0_d[:, :, :], out_b0)

            # window 1, half A (frames 8-11) then half B (12-15)
            for h, OB1 in ((0, OB1a), (1, OB1b)):
                hs = slice(4 * h, 4 * h + 4)
                V.reduce_max(
                    maxo[:, 1, :, hs],
                    OB1.rearrange("p c (f o) -> p c f o", o=NO),
                    axis=mybir.AxisListType.X,
                )
                V.tensor_scalar_mul(maxo[:, 1, :, hs], maxo[:, 1, :, hs], 1.0 / WOB_SCALE)
                for m2 in range(2):
                    obj_add(1, m2, hs, 128 + 64 * h, 4, out_b1, 64 * h)
            nc.scalar.dma_start(out_b1_d[:, :, :], out_b1)

    nc.compile()
    return nc


def _get_nc():
    global _NC_CACHE
    if _NC_CACHE is None:
        _NC_CACHE = _build_nc()
    return _NC_CACHE


def _marshal(pf, of, Wp, bp, Wpr, bpr, Wo, bo, Wm_obj, bm_obj, Wm_per, bm_per):
    """Pack full f32 inputs into per-core DRAM parameter layouts."""
    pf_bf = pf.astype(BF16)
    of_q = of.astype(FP8)

    Wab = Wpr @ Wm_per[C:]                                               # [D, C] fused BP weight
    Wob = Wo @ Wm_obj[C:]                                                # [D, C] fused OB weight
    wp_packed = Wp.astype(BF16).reshape(KD, 128, 256).transpose(1, 0, 2)  # [128, KD, 256]
    wab_packed = (Wab * WAB_SCALE).astype(FP8).reshape(KD, 128, 256).transpose(1, 0, 2)
    wob_packed = (Wob * WOB_SCALE).astype(FP8).reshape(KD, 128, 256).transpose(1, 0, 2)
    wmcat = np.concatenate([Wm_obj[:C], Wm_per[:C]], axis=1).astype(BF16)  # [C, 512]
    wm_packed = wmcat.reshape(2, 128, 512).transpose(1, 0, 2)            # [128, 2, 512]

    bias_obj = bm_obj + bp @ Wm_obj[:C] + bo @ Wm_obj[C:]
    bias_per = bm_per + bp @ Wm_per[:C] + bpr @ Wm_per[C:]
    # bias rides in wm as two extra bf16 columns: row 0 obj halves, row 1 per
    bias4 = np.stack(
        [bias_obj[0:128], bias_obj[128:256], bias_per[0:128], bias_per[128:256]],
        axis=1,
    ).astype(BF16)                                                       # [128, 4]
    wmb = np.concatenate([wm_packed, bias4.reshape(128, 2, 2)], axis=2)  # [128, 2, 514]
    wmb = np.ascontiguousarray(wmb)

    in_maps = []
    for c in range(NCORES):
        pfc = pf_bf[c * TP : (c + 1) * TP]                                # [TP, D]
        ofc = of_q[c * TO : (c + 1) * TO]                                 # [TO, D]
        pf_packed = pfc.reshape(TP, KD, 128).transpose(2, 1, 0)           # [128, KD, TP]
        a_full = np.concatenate([wp_packed, pf_packed], axis=2)           # [128, KD, 512]
        of_packed = ofc.reshape(TO, KD, 128).transpose(2, 1, 0)           # [128, KD, TO]
        owcat = np.concatenate(
            [wob_packed, of_packed[:, :, 0:W0]], axis=2
        )                                                                 # [128, KD, 640]
        m = {"wm": wmb, "wab8": wab_packed}
        for i in range(5):
            m[f"a{i}"] = np.ascontiguousarray(
                a_full[:, A_START[i] : A_START[i] + A_SPLIT[i], :]
            )
        for h, (k0, kn) in enumerate(((0, 8), (8, 4), (12, 4))):
            m[f"ow{h}"] = np.ascontiguousarray(owcat[:, k0 : k0 + kn, :])
            m[f"ow1{h}"] = np.ascontiguousarray(
                of_packed[:, k0 : k0 + kn, W0:TO]
            )
        in_maps.append(m)
    return in_maps


def _unmarshal(results):
    """Per-core 3-piece bf16 output -> [F*NP, 2C, 1,1,1] f32."""
    blocks = []
    for c in range(NCORES):
        r = results[c]
        obj = np.concatenate(
            [np.asarray(r["out_b0"]), np.asarray(r["out_b1"])], axis=2
        ).astype(np.float32)                                              # [128, 2, TP]
        per = np.asarray(r["out_p"]).astype(np.float32)                   # [128, 2, TP]
        arr = np.concatenate([obj, per], axis=1)                          # [128, 4, TP]
        out_t = arr.transpose(1, 0, 2).reshape(2 * C, TP)                 # [512, TP]
        blocks.append(out_t.T)                                            # [TP, 512]
    full = np.concatenate(blocks, axis=0).astype(np.float32)              # [F*NP, 2C]
    return full[:, :, None, None, None]


def kernel(
    person_feature,
    obj_feature,
    Wp,
    bp,
    Wpr,
    bpr,
    Wo,
    bo,
    Wm_obj,
    bm_obj,
    Wm_per,
    bm_per,
    f_num,
    np_pf,
    no_pf,
):
    assert int(f_num) == F and int(np_pf) == NP and int(no_pf) == NO
    pf = np.asarray(person_feature, dtype=np.float32)[:, :, 0, 0, 0]
    of = np.asarray(obj_feature, dtype=np.float32)[:, :, 0, 0, 0]
    args = [
        np.asarray(a, dtype=np.float32)
        for a in (Wp, bp, Wpr, bpr, Wo, bo, Wm_obj, bm_obj, Wm_per, bm_per)
    ]
    in_maps = _marshal(pf, of, *args)
    nc = _get_nc()
    res = run_bass_kernel_spmd(nc, in_maps, core_ids=list(range(NCORES)))
    return _unmarshal(res.results)


if __name__ == "__main__":
    # smoke test with random data against a numpy re-derivation
    rng = np.random.default_rng(0)
    pf = rng.standard_normal((F * NP, D, 1, 1, 1), dtype=np.float32)
    of = rng.standard_normal((F * NO, D, 1, 1, 1), dtype=np.float32)
    mk = lambda *s: (rng.standard_normal(s, dtype=np.float32) * 0.01)
    inputs = dict(
        person_feature=pf,
        obj_feature=of,
        Wp=mk(D, C),
        bp=np.zeros(C, np.float32),
        Wpr=mk(D, C),
        bpr=np.zeros(C, np.float32),
        Wo=mk(D, C),
        bo=np.zeros(C, np.float32),
        Wm_obj=rng.standard_normal((2 * C, C), dtype=np.float32) / np.sqrt(2 * C),
        bm_obj=np.zeros(C, np.float32),
        Wm_per=rng.standard_normal((2 * C, C), dtype=np.float32) / np.sqrt(2 * C),
        bm_per=np.zeros(C, np.float32),
        f_num=F,
        np_pf=NP,
        no_pf=NO,
    )
    out = kernel(**inputs)
    print("kernel output shape:", out.shape)


# revision 45
# speedup vs baseline: 1.0726x; 1.0085x over previous
"""Trainium2 Bass kernel for nn_AOGStructure (gnn_message_passing).

Reference computation (per frame f, with NP persons / NO objects, C=256):
    P = pf @ Wp + bp            # persons_red
    A = pf @ Wpr + bpr          # act_persons_red
    O = of @ Wo + bo            # objs_red
    objs_interact[f,i]    = max_j       (P[f,i] @ Wm_obj[:C] + O[f,j] @ Wm_obj[C:] + bm_obj)
    persons_interact[f,i] = max_{j!=i}  (P[f,i] @ Wm_per[:C] + A[f,j] @ Wm_per[C:] + bm_per)
    out = concat([objs_interact, persons_interact], -1)

Since the per-pair message is additive in (i-term, j-term), the max over j
factorizes:  max_j (a_i + b_j) = a_i + max_j b_j.  The [F,NP,NO,C] pair tensor
is never materialized.  For the person block the self-excluded max is computed
from the max and the masked ("second") max.  All biases commute with the max
and are folded into a single per-output-channel bias vector added at the end.

Strategy: data-parallel over frames, 16 frames per core, weights replicated,
no collectives.  A single DMA stream whose transfer order equals PE
consumption order, issued as ~15 large contiguous DMAs (per-DMA issue costs
~0.65us on the SP queue, so small transfers are ruinous):

  phase A   5 chunks of [wpa_k | pf_k]   -> BP/P matmuls   (bf16)
  wm/bias                                -> stage 2 + epilogues
  phase B   4x [wob_g | of_w0_g]         -> OB window-0    (fp8 DoubleRow)
  phase C   4x of_w1_g                   -> OB window-1 (two half-windows)

The whole object path runs in fp8-e4m3: `of` quantized directly, Wob
pre-scaled by 2048 (73% of Wob underflows into e4m3 subnormals unscaled) and
the 1/2048 folded into the per-window max fixup.  Both operands fp8 enables
MatmulPerfMode.DoubleRow: two contraction rows per PE cycle, halving the OB
phase.

The PE HAM clock gate defaults to 1.2GHz and only reaches 2.4GHz after
~3.4us of *continuous* matmul activity; any idle gap restarts the ramp.  So
junk matmuls (into the not-yet-live OB1b PSUM bank) fill every DMA-wait gap:
a burst before phase A and a small burst before each chunk-gated LDWEIGHTS.
Within phase A the BP matmuls run before P per chunk so BP retires first and
the person epilogue (the long DVE chain) starts as early as possible.

Epilogues are split across engines: DVE owns PSUM reads (reduce_max) and the
m2=0 half of the person chain; Pool (gpsimd) gets the m2=1 half plus all
SBUF-only object fixups.  PSUM->SBUF staging copies run on Scalar.  Output is
bf16 in three params (person half / obj window-0 / obj window-1), each DMA'd
on the Scalar queue as soon as its last add lands, so the final transfer is a
64KB sliver instead of the whole output.  PSUM uses exactly 8 banks.
"""

import sys

if "/opt/trn_rl_repo" not in sys.path:
    sys.path.insert(0, "/opt/trn_rl_repo")

import ml_dtypes
import numpy as np

import concourse.bass as bass  # noqa: F401  (import keeps bass registered)
import concourse.tile as tile
from concourse import bacc, mybir
from concourse.bass_utils import run_bass_kernel_spmd

NCORES = 8
F, NP, NO = 128, 16, 48
D, C = 2048, 256
F_LOC = F // NCORES          # 16 frames per core
TP = F_LOC * NP              # 256 person tokens per core
TO = F_LOC * NO              # 768 object tokens per core
KD = D // 128                # 16 contraction chunks of 128
W0 = 384                     # of window 0: frames 0-7
F0 = W0 // NO                # 8 frames in window 0
WOB_SCALE = 2048.0           # keeps fp8 Wob out of the subnormal range
BF16 = ml_dtypes.bfloat16
FP8 = ml_dtypes.float8_e4m3

# k-extents of the five phase-A chunks (first small so the PE starts early)
A_SPLIT = [1, 3, 4, 4, 4]
A_START = [0, 1, 4, 8, 12]
N_WARMUP = 12                # junk matmuls to ramp the PE clock before data
WAB_SCALE = 128.0            # keeps fp8 Wab out of the subnormal range

_NC_CACHE = None


def _build_nc():
    """Build the single-core SPMD graph (same NEFF on all 8 cores)."""
    nc = bacc.Bacc("TRN2", target_bir_lowering=False, debug=False)
    BF = mybir.dt.bfloat16
    F8 = mybir.dt.float8e4
    F32 = mybir.dt.float32
    DR = mybir.MatmulPerfMode.DoubleRow

    a_d = [
        nc.declare_dram_parameter(f"a{i}", [128, A_SPLIT[i], 512], BF, isOutput=False)
        for i in range(5)
    ]
    # fused BP weight ships as fp8 (x128 to clear the subnormal range) and is
    # consumed DIRECTLY as the fp8 stationary operand against bf16 pf (the PE
    # allows mixed non-fp32 operand dtypes; HW-verified exact) — half the HBM
    # bytes of a bf16 Wab at identical matmul cost
    wab_d = nc.declare_dram_parameter("wab8", [128, KD, 256], F8, isOutput=False)
    # merged fp8 chunks, k-split 8/4/4: per k-row [wob_k (256) | of_w0_k (384)]
    OW_SPLIT = [8, 4, 4]
    OW_START = [0, 8, 12]
    ow_d = [
        nc.declare_dram_parameter(f"ow{h}", [128, OW_SPLIT[h], 640], F8, isOutput=False)
        for h in range(3)
    ]
    # of window 1, k-split 8/4/4 so the final DMA chunk (and the matmul work
    # gated on it) is as small as possible
    OW1_SPLIT = [8, 4, 4]
    OW1_START = [0, 8, 12]
    ow1_d = [
        nc.declare_dram_parameter(f"ow1{h}", [128, OW1_SPLIT[h], W0], F8, isOutput=False)
        for h in range(3)
    ]
    # wm plus the four bias vectors as two extra bf16 columns per row
    wm_d = nc.declare_dram_parameter("wm", [128, 2, 514], BF, isOutput=False)
    # output in three pieces so each can DMA out as soon as it completes:
    # op = person half [c2,c3], ob0 = obj window-0 frames, ob1 = obj window-1
    out_p_d = nc.declare_dram_parameter("out_p", [128, 2, TP], BF, isOutput=True)
    out_b0_d = nc.declare_dram_parameter("out_b0", [128, 2, F0 * NP], BF, isOutput=True)
    out_b1_d = nc.declare_dram_parameter("out_b1", [128, 2, TP - F0 * NP], BF, isOutput=True)

    with tile.TileContext(nc) as tc:
        with (
            tc.tile_pool(name="loads", bufs=1) as loads,
            tc.tile_pool(name="work", bufs=1) as work,
            tc.tile_pool(name="psum", bufs=8, space="PSUM") as psum,
        ):
            # ---- input DMAs on the SP queue, in PE consumption order.  The
            #      fp8 wab rides as ONE transfer after a1 — many small
            #      transfers (sub-512B per partition line) starve the stream
            #      on issue cost and RMW penalties. ----
            a_sb = []
            wab_sb = None
            for i in range(5):
                t = loads.tile([128, A_SPLIT[i], 512], BF, tag=f"a{i}", name=f"a{i}")
                nc.sync.dma_start(t, a_d[i][:, :, :])
                a_sb.append(t)
                if i == 1:
                    wab_sb = loads.tile([128, KD, 256], F8, tag="wab", name="wab")
                    nc.sync.dma_start(wab_sb, wab_d[:, :, :])
            wm_sb = loads.tile([128, 2, 514], BF, tag="wm", name="wm")
            nc.sync.dma_start(wm_sb, wm_d[:, :, :])
            ow_sb = [None] * 3
            ow1_sb = [None] * 3
            for h in range(3):
                t = loads.tile([128, OW_SPLIT[h], 640], F8, tag=f"ow{h}", name=f"ow{h}")
                nc.sync.dma_start(t, ow_d[h][:, :, :])
                ow_sb[h] = t
            for h in range(3):
                t = loads.tile([128, OW1_SPLIT[h], W0], F8, tag=f"ow1{h}", name=f"ow1{h}")
                nc.sync.dma_start(t, ow1_d[h][:, :, :])
                ow1_sb[h] = t

            def achunk(k):
                for i in range(4, -1, -1):
                    if k >= A_START[i]:
                        return a_sb[i], k - A_START[i]
                raise AssertionError

            def wpchunk(k, m):  # Wp chunk (feeds P)
                t, kk = achunk(k)
                return t[:, kk, m * 128 : m * 128 + 128]

            def wabchunk(k, m):  # Wab chunk (fp8 stationary, feeds BP)
                return wab_sb[:, k, m * 128 : m * 128 + 128]

            def pfchunk(k):
                t, kk = achunk(k)
                return t[:, kk, 256:512]

            def wmchunk(kc, sec, m2):  # sec 0 = a_o (Wm1a), 1 = a_p (Wm2a)
                j0 = sec * 256 + m2 * 128
                return wm_sb[:, kc, j0 : j0 + 128]

            def _owc(g, kk):
                k = g * 4 + kk
                c = 0 if k < 8 else (1 if k < 12 else 2)
                return ow_sb[c], k - OW_START[c]

            def wobpair(g, kk, m2):  # [128, 2, 128] fp8 stationary, k-pair
                t, r = _owc(g, kk)
                return t[:, r : r + 2, m2 * 128 : m2 * 128 + 128]

            def ow0pair(g, kk):
                t, r = _owc(g, kk)
                return t[:, r : r + 2, 256:640]

            def ow1pair(g, kk, lo):
                k = g * 4 + kk
                c = 0 if k < 8 else (1 if k < 12 else 2)
                r = k - OW1_START[c]
                return ow1_sb[c][:, r : r + 2, lo : lo + 192]

            # bias views packed into wm: row 0 = object halves, row 1 = person
            def bias_obj(m2):  # [128, 1]
                return wm_sb[:, 0, 512 + m2 : 513 + m2]

            bias_per = wm_sb[:, 1, 512:514]  # [128, 2]

            # ---- PSUM: exactly 8 banks ----
            P_ps = psum.tile([128, 2, TP], F32, tag="ps", name="P_ps")
            BP_ps = psum.tile([128, 2, TP], F32, tag="ps", name="BP_ps")
            AP_ps = psum.tile([128, 2, TP], F32, tag="ps", name="AP_ps")
            AO_ps = psum.tile([128, 2, TP], F32, tag="ps", name="AO_ps")
            OB0 = [psum.tile([128, W0], F32, tag="ps", name=f"OB0_{m2}") for m2 in range(2)]
            OB1a = psum.tile([128, 2, 192], F32, tag="ps", name="OB1a")
            OB1b = psum.tile([128, 2, 192], F32, tag="ps", name="OB1b")

            # ---- PE warmup: the HAM clock gate needs ~3.4us of CONTINUOUS
            #      matmul activity to unthrottle 1.2 -> 2.4GHz, so junk
            #      matmuls fill the pre-data window AND every DMA-wait gap
            #      between phase-A chunks.  Mid-phase junk lands in the OB1b
            #      bank, which has no live accumulation until OB window 1
            #      (whose first real matmul has start=True and clears it). ----
            junk = work.tile([128, 256], BF, tag="junk", name="junk")
            nc.gpsimd.memset(junk, 0)

            def junk_mm(n, tgt=None, ncols=256):
                for _ in range(n):
                    nc.tensor.matmul(
                        tgt if tgt is not None else P_ps[:, 0, :],
                        junk[:, 0:128], junk[:, 0:ncols],
                        start=True, stop=True, skip_group_check=True,
                    )

            junk_mm(N_WARMUP)

            # ---- phase A, paced by a-chunk arrival.  Per chunk, P matmuls
            #      run first (their weights ride the a-chunks); BP trails by
            #      two chunks so the single wab transfer has landed.  Junk
            #      bursts bridge the pre-warm DMA gaps only — once the HAM
            #      clock is warm, short idles cost nothing.  The last chunk
            #      runs BP-first, m-half-major, so the PSUM->SBUF copies (and
            #      everything gated on them) pipeline with the remaining
            #      matmuls.  Mid-phase junk lands in OB1b's bank (no live
            #      accumulation there until OB window 1, whose first matmul
            #      clears it). ----
            def p_mms(ci, order):
                ks = range(A_START[ci], A_START[ci] + A_SPLIT[ci])
                for m, k in order(ks):
                    nc.tensor.matmul(P_ps[:, m, :], wpchunk(k, m), pfchunk(k),
                                     start=(k == 0 and m == 0),
                                     stop=(k == KD - 1))

            def bp_mms(ci, order):
                ks = range(A_START[ci], A_START[ci] + A_SPLIT[ci])
                for m, k in order(ks):
                    nc.tensor.matmul(BP_ps[:, m, :], wabchunk(k, m), pfchunk(k),
                                     start=(k == 0 and m == 0),
                                     stop=(k == KD - 1))

            k_major = lambda ks: [(m, k) for k in ks for m in range(2)]
            m_major = lambda ks: [(m, k) for m in range(2) for k in ks]
            p_mms(0, k_major)
            junk_mm(4, tgt=OB1b[:, 0, :], ncols=192)
            p_mms(1, k_major)
            junk_mm(3, tgt=OB1b[:, 0, :], ncols=192)
            bp_mms(0, k_major)
            bp_mms(1, k_major)
            for ci in (2, 3):
                p_mms(ci, k_major)
                bp_mms(ci, k_major)
            bp_mms(4, m_major)
            p_mms(4, m_major)

            # BP/P PSUM -> SBUF per m-half on Scalar, in retirement order, so
            # the person epilogue and stage 2 start as early as possible.  The
            # BP copy folds the 1/WAB_SCALE descale.
            CP = mybir.ActivationFunctionType.Copy
            BPc = work.tile([128, 2, TP], BF, tag="BPc", name="BPc")
            PT = work.tile([128, 2, TP], BF, tag="PTsb", name="PTsb")
            for m in range(2):
                nc.scalar.activation(BPc[:, m, :], BP_ps[:, m, :], CP,
                                     scale=1.0 / WAB_SCALE)
            for m in range(2):
                nc.scalar.copy(PT[:, m, :], P_ps[:, m, :])

            # ---- stage 2: a_p (AP) and a_o (AO) from PT, kc-outer so the
            #      kc=0 matmuls only wait on PT's first half ----
            for kc in range(2):
                for m2 in range(2):
                    st, sp = (kc == 0 and m2 == 0), (kc == 1)
                    nc.tensor.matmul(AP_ps[:, m2, :], wmchunk(kc, 1, m2), PT[:, kc, :], start=st, stop=sp)
                    nc.tensor.matmul(AO_ps[:, m2, :], wmchunk(kc, 0, m2), PT[:, kc, :], start=st, stop=sp)

            # ---- OB window 0: fp8 DoubleRow, two k-planes per matmul ----
            for g in range(4):
                for kk in (0, 2):
                    k = g * 4 + kk
                    for m2 in range(2):
                        nc.tensor.matmul(
                            OB0[m2], wobpair(g, kk, m2),
                            ow0pair(g, kk),
                            start=(k == 0), stop=(k == KD - 2),
                            perf_mode=DR,
                        )

            # ---- remaining PSUM -> SBUF copies (Scalar) ----
            APc = work.tile([128, 2, TP], BF, tag="APc", name="APc")
            nc.scalar.copy(APc, AP_ps)
            AOc = work.tile([128, 2, TP], BF, tag="AOc", name="AOc")
            nc.scalar.copy(AOc, AO_ps)

            # ---- persons epilogue (self-excluded max), bf16 on DVE
            # (free-axis reductions are DVE-only on TRN2; bf16 doubles DVE
            # throughput) ----
            SH3, SH4 = (128, 2, F_LOC), (128, 2, F_LOC, NP)
            out_p = work.tile([128, 2, TP], BF, tag="out_p", name="out_p")
            out_b0 = work.tile([128, 2, F0 * NP], BF, tag="out_b0", name="out_b0")
            out_b1 = work.tile([128, 2, TP - F0 * NP], BF, tag="out_b1", name="out_b1")
            V = nc.vector
            G = nc.gpsimd
            bp4 = BPc.rearrange("p c (f i) -> p c f i", i=NP)
            m1 = work.tile(list(SH3), BF, tag="m1", name="m1")
            V.reduce_max(m1, bp4, axis=mybir.AxisListType.X)
            eq = work.tile(list(SH4), BF, tag="eq", name="eq")
            V.tensor_tensor(eq, bp4, m1[:, :, :, None].to_broadcast(SH4),
                            mybir.AluOpType.is_equal)
            msk = work.tile(list(SH4), BF, tag="msk", name="msk")
            V.scalar_tensor_tensor(msk, eq, -1e30, bp4,
                                   mybir.AluOpType.mult, mybir.AluOpType.add)
            m2v = work.tile(list(SH3), BF, tag="m2v", name="m2v")
            V.reduce_max(m2v, msk, axis=mybir.AxisListType.X)
            # the whole mex chain runs on Pool: it is ~2x slower per element
            # there, but it is fully hidden behind the OB matmuls, and it
            # keeps the DVE FIFO free for the object reduces + adds (in v5
            # the w1b reduce sat 3.3us behind these ops in the DVE queue)
            dd = work.tile(list(SH3), BF, tag="dd", name="dd")
            G.tensor_tensor(dd, m2v, m1, mybir.AluOpType.subtract)
            m1pb = work.tile(list(SH3), BF, tag="m1pb", name="m1pb")
            G.tensor_tensor(m1pb, m1, bias_per[:, :, None].to_broadcast(SH3),
                            mybir.AluOpType.add)
            mex = work.tile(list(SH4), BF, tag="mex", name="mex")
            G.tensor_tensor(mex, eq, dd[:, :, :, None].to_broadcast(SH4),
                            mybir.AluOpType.mult)
            G.tensor_tensor(mex, mex, m1pb[:, :, :, None].to_broadcast(SH4),
                            mybir.AluOpType.add)
            G.tensor_tensor(
                out_p.rearrange("p c (f i) -> p c f i", i=NP),
                APc.rearrange("p c (f i) -> p c f i", i=NP),
                mex, mybir.AluOpType.add,
            )
            nc.scalar.dma_start(out_p_d[:, :, :], out_p)

            # ---- OB window 1: k-pair outer, both half-windows inner, so each
            #      incoming ow1 chunk is fully consumed before the next is
            #      needed and the post-last-chunk matmul tail is 8 MMs.  The
            #      a-half's stop lands before the b-half's, so its reduce
            #      overlaps the b-half's final matmuls. ----
            for g in range(4):
                for kk in (0, 2):
                    k = g * 4 + kk
                    for OB1, lo in ((OB1a, 0), (OB1b, 192)):
                        for m2 in range(2):
                            nc.tensor.matmul(
                                OB1[:, m2, :], wobpair(g, kk, m2),
                                ow1pair(g, kk, lo),
                                start=(k == 0 and m2 == 0),
                                stop=(k == KD - 2),
                                perf_mode=DR,
                            )

            # ---- object epilogues ----
            # Reduces straight from PSUM on DVE (the only engine that can),
            # a cheap descale on the tiny maxo tile, then one fused
            # scalar_tensor_tensor per (window, channel-half) — all DVE.
            # With the person mex chain on Pool, the DVE FIFO here is short.
            maxo = work.tile([128, 2, 2, F0], F32, tag="maxo", name="maxo")

            def obj_add(w, m2, hslc, t0, nfr, dst, d0):
                V.scalar_tensor_tensor(
                    dst[:, m2, d0 : d0 + nfr * NP].rearrange(
                        "p (f i) -> p f i", i=NP
                    ),
                    maxo[:, w, m2, hslc, None].to_broadcast((128, nfr, NP)),
                    bias_obj(m2),
                    AOc[:, m2, t0 : t0 + nfr * NP].rearrange("p (f i) -> p f i", i=NP),
                    mybir.AluOpType.add,
                    mybir.AluOpType.add,
                )

            # window 0 (frames 0-7), one reduce per OB0 psum bank
            for m2 in range(2):
                V.reduce_max(
                    maxo[:, 0, m2, :],
                    OB0[m2].rearrange("p (f o) -> p f o", o=NO),
                    axis=mybir.AxisListType.X,
                )
            V.tensor_scalar_mul(maxo[:, 0, :, :], maxo[:, 0, :, :], 1.0 / WOB_SCALE)
            for m2 in range(2):
                obj_add(0, m2, slice(0, F0), 0, F0, out_b0, 0)
            nc.scalar.dma_start(out_b0_d[:, :, :], out_b0)

            # window 1, half A (frames 8-11) then half B (12-15)
            for h, OB1 in ((0, OB1a), (1, OB1b)):
                hs = slice(4 * h, 4 * h + 4)
                V.reduce_max(
                    maxo[:, 1, :, hs],
                    OB1.rearrange("p c (f o) -> p c f o", o=NO),
                    axis=mybir.AxisListType.X,
                )
                V.tensor_scalar_mul(maxo[:, 1, :, hs], maxo[:, 1, :, hs], 1.0 / WOB_SCALE)
                for m2 in range(2):
                    obj_add(1, m2, hs, 128 + 64 * h, 4, out_b1, 64 * h)
            nc.scalar.dma_start(out_b1_d[:, :, :], out_b1)

    nc.compile()
    return nc


def _get_nc():
    global _NC_CACHE
    if _NC_CACHE is None:
        _NC_CACHE = _build_nc()
    return _NC_CACHE


def _marshal(pf, of, Wp, bp, Wpr, bpr, Wo, bo, Wm_obj, bm_obj, Wm_per, bm_per):
    """Pack full f32 inputs into per-core DRAM parameter layouts."""
    pf_bf = pf.astype(BF16)
    of_q = of.astype(FP8)

    Wab = Wpr @ Wm_per[C:]                                               # [D, C] fused BP weight
    Wob = Wo @ Wm_obj[C:]                                                # [D, C] fused OB weight
    wp_packed = Wp.astype(BF16).reshape(KD, 128, 256).transpose(1, 0, 2)  # [128, KD, 256]
    wab_packed = (Wab * WAB_SCALE).astype(FP8).reshape(KD, 128, 256).transpose(1, 0, 2)
    wob_packed = (Wob * WOB_SCALE).astype(FP8).reshape(KD, 128, 256).transpose(1, 0, 2)
    wmcat = np.concatenate([Wm_obj[:C], Wm_per[:C]], axis=1).astype(BF16)  # [C, 512]
    wm_packed = wmcat.reshape(2, 128, 512).transpose(1, 0, 2)            # [128, 2, 512]

    bias_obj = bm_obj + bp @ Wm_obj[:C] + bo @ Wm_obj[C:]
    bias_per = bm_per + bp @ Wm_per[:C] + bpr @ Wm_per[C:]
    # bias rides in wm as two extra bf16 columns: row 0 obj halves, row 1 per
    bias4 = np.stack(
        [bias_obj[0:128], bias_obj[128:256], bias_per[0:128], bias_per[128:256]],
        axis=1,
    ).astype(BF16)                                                       # [128, 4]
    wmb = np.concatenate([wm_packed, bias4.reshape(128, 2, 2)], axis=2)  # [128, 2, 514]
    wmb = np.ascontiguousarray(wmb)

    in_maps = []
    for c in range(NCORES):
        pfc = pf_bf[c * TP : (c + 1) * TP]                                # [TP, D]
        ofc = of_q[c * TO : (c + 1) * TO]                                 # [TO, D]
        pf_packed = pfc.reshape(TP, KD, 128).transpose(2, 1, 0)           # [128, KD, TP]
        a_full = np.concatenate([wp_packed, pf_packed], axis=2)           # [128, KD, 512]
        of_packed = ofc.reshape(TO, KD, 128).transpose(2, 1, 0)           # [128, KD, TO]
        owcat = np.concatenate(
            [wob_packed, of_packed[:, :, 0:W0]], axis=2
        )                                                                 # [128, KD, 640]
        m = {"wm": wmb, "wab8": wab_packed}
        for i in range(5):
            m[f"a{i}"] = np.ascontiguousarray(
                a_full[:, A_START[i] : A_START[i] + A_SPLIT[i], :]
            )
        for h, (k0, kn) in enumerate(((0, 8), (8, 4), (12, 4))):
            m[f"ow{h}"] = np.ascontiguousarray(owcat[:, k0 : k0 + kn, :])
            m[f"ow1{h}"] = np.ascontiguousarray(
                of_packed[:, k0 : k0 + kn, W0:TO]
            )
        in_maps.append(m)
    return in_maps


def _unmarshal(results):
    """Per-core 3-piece bf16 output -> [F*NP, 2C, 1,1,1] f32."""
    blocks = []
    for c in range(NCORES):
        r = results[c]
        obj = np.concatenate(
            [np.asarray(r["out_b0"]), np.asarray(r["out_b1"])], axis=2
        ).astype(np.float32)                                              # [128, 2, TP]
        per = np.asarray(r["out_p"]).astype(np.float32)                   # [128, 2, TP]
        arr = np.concatenate([obj, per], axis=1)                          # [128, 4, TP]
        out_t = arr.transpose(1, 0, 2).reshape(2 * C, TP)                 # [512, TP]
        blocks.append(out_t.T)                                            # [TP, 512]
    full = np.concatenate(blocks, axis=0).astype(np.float32)              # [F*NP, 2C]
    return full[:, :, None, None, None]


def kernel(
    person_feature,
    obj_feature,
    Wp,
    bp,
    Wpr,
    bpr,
    Wo,
    bo,
    Wm_obj,
    bm_obj,
    Wm_per,
    bm_per,
    f_num,
    np_pf,
    no_pf,
):
    assert int(f_num) == F and int(np_pf) == NP and int(no_pf) == NO
    pf = np.asarray(person_feature, dtype=np.float32)[:, :, 0, 0, 0]
    of = np.asarray(obj_feature, dtype=np.float32)[:, :, 0, 0, 0]
    args = [
        np.asarray(a, dtype=np.float32)
        for a in (Wp, bp, Wpr, bpr, Wo, bo, Wm_obj, bm_obj, Wm_per, bm_per)
    ]
    in_maps = _marshal(pf, of, *args)
    nc = _get_nc()
    res = run_bass_kernel_spmd(nc, in_maps, core_ids=list(range(NCORES)))
    return _unmarshal(res.results)


if __name__ == "__main__":
    # smoke test with random data against a numpy re-derivation
    rng = np.random.default_rng(0)
    pf = rng.standard_normal((F * NP, D, 1, 1, 1), dtype=np.float32)
    of = rng.standard_normal((F * NO, D, 1, 1, 1), dtype=np.float32)
    mk = lambda *s: (rng.standard_normal(s, dtype=np.float32) * 0.01)
    inputs = dict(
        person_feature=pf,
        obj_feature=of,
        Wp=mk(D, C),
        bp=np.zeros(C, np.float32),
        Wpr=mk(D, C),
        bpr=np.zeros(C, np.float32),
        Wo=mk(D, C),
        bo=np.zeros(C, np.float32),
        Wm_obj=rng.standard_normal((2 * C, C), dtype=np.float32) / np.sqrt(2 * C),
        bm_obj=np.zeros(C, np.float32),
        Wm_per=rng.standard_normal((2 * C, C), dtype=np.float32) / np.sqrt(2 * C),
        bm_per=np.zeros(C, np.float32),
        f_num=F,
        np_pf=NP,
        no_pf=NO,
    )
    out = kernel(**inputs)
    print("kernel output shape:", out.shape)


# revision 47
# speedup vs baseline: 1.0784x; 1.0054x over previous
"""Trainium2 Bass kernel for nn_AOGStructure (gnn_message_passing).

Reference computation (per frame f, with NP persons / NO objects, C=256):
    P = pf @ Wp + bp            # persons_red
    A = pf @ Wpr + bpr          # act_persons_red
    O = of @ Wo + bo            # objs_red
    objs_interact[f,i]    = max_j       (P[f,i] @ Wm_obj[:C] + O[f,j] @ Wm_obj[C:] + bm_obj)
    persons_interact[f,i] = max_{j!=i}  (P[f,i] @ Wm_per[:C] + A[f,j] @ Wm_per[C:] + bm_per)
    out = concat([objs_interact, persons_interact], -1)

Since the per-pair message is additive in (i-term, j-term), the max over j
factorizes:  max_j (a_i + b_j) = a_i + max_j b_j.  The [F,NP,NO,C] pair tensor
is never materialized.  For the person block the self-excluded max is computed
from the max and the masked ("second") max.  All biases commute with the max
and are folded into a single per-output-channel bias vector added at the end.

Strategy: data-parallel over frames, 16 frames per core, weights replicated,
no collectives.  A single DMA stream whose transfer order equals PE
consumption order, issued as ~15 large contiguous DMAs (per-DMA issue costs
~0.65us on the SP queue, so small transfers are ruinous):

  phase A   5 chunks of [wpa_k | pf_k]   -> BP/P matmuls   (bf16)
  wm/bias                                -> stage 2 + epilogues
  phase B   4x [wob_g | of_w0_g]         -> OB window-0    (fp8 DoubleRow)
  phase C   4x of_w1_g                   -> OB window-1 (two half-windows)

The whole object path runs in fp8-e4m3: `of` quantized directly, Wob
pre-scaled by 2048 (73% of Wob underflows into e4m3 subnormals unscaled) and
the 1/2048 folded into the per-window max fixup.  Both operands fp8 enables
MatmulPerfMode.DoubleRow: two contraction rows per PE cycle, halving the OB
phase.

The PE HAM clock gate defaults to 1.2GHz and only reaches 2.4GHz after
~3.4us of *continuous* matmul activity; any idle gap restarts the ramp.  So
junk matmuls (into the not-yet-live OB1b PSUM bank) fill every DMA-wait gap:
a burst before phase A and a small burst before each chunk-gated LDWEIGHTS.
Within phase A the BP matmuls run before P per chunk so BP retires first and
the person epilogue (the long DVE chain) starts as early as possible.

Epilogues are split across engines: DVE owns PSUM reads (reduce_max) and the
m2=0 half of the person chain; Pool (gpsimd) gets the m2=1 half plus all
SBUF-only object fixups.  PSUM->SBUF staging copies run on Scalar.  Output is
bf16 in three params (person half / obj window-0 / obj window-1), each DMA'd
on the Scalar queue as soon as its last add lands, so the final transfer is a
64KB sliver instead of the whole output.  PSUM uses exactly 8 banks.
"""

import sys

if "/opt/trn_rl_repo" not in sys.path:
    sys.path.insert(0, "/opt/trn_rl_repo")

import ml_dtypes
import numpy as np

import concourse.bass as bass  # noqa: F401  (import keeps bass registered)
import concourse.tile as tile
from concourse import bacc, mybir
from concourse.bass_utils import run_bass_kernel_spmd

NCORES = 8
F, NP, NO = 128, 16, 48
D, C = 2048, 256
F_LOC = F // NCORES          # 16 frames per core
TP = F_LOC * NP              # 256 person tokens per core
TO = F_LOC * NO              # 768 object tokens per core
KD = D // 128                # 16 contraction chunks of 128
W0 = 384                     # of window 0: frames 0-7
F0 = W0 // NO                # 8 frames in window 0
WOB_SCALE = 2048.0           # keeps fp8 Wob out of the subnormal range
BF16 = ml_dtypes.bfloat16
FP8 = ml_dtypes.float8_e4m3

# k-extents of the five phase-A chunks (first small so the PE starts early)
A_SPLIT = [1, 3, 4, 4, 4]
A_START = [0, 1, 4, 8, 12]
N_WARMUP = 12                # junk matmuls to ramp the PE clock before data
WAB_SCALE = 128.0            # keeps fp8 Wab out of the subnormal range

_NC_CACHE = None


def _build_nc():
    """Build the single-core SPMD graph (same NEFF on all 8 cores)."""
    nc = bacc.Bacc("TRN2", target_bir_lowering=False, debug=False)
    BF = mybir.dt.bfloat16
    F8 = mybir.dt.float8e4
    F32 = mybir.dt.float32
    DR = mybir.MatmulPerfMode.DoubleRow

    a_d = [
        nc.declare_dram_parameter(f"a{i}", [128, A_SPLIT[i], 512], BF, isOutput=False)
        for i in range(5)
    ]
    # fused BP weight ships as fp8 (x128 to clear the subnormal range) and is
    # consumed DIRECTLY as the fp8 stationary operand against bf16 pf (the PE
    # allows mixed non-fp32 operand dtypes; HW-verified exact) — half the HBM
    # bytes of a bf16 Wab at identical matmul cost
    wab_d = nc.declare_dram_parameter("wab8", [128, KD, 256], F8, isOutput=False)
    # merged fp8 chunks, k-split 8/4/4: per k-row [wob_k (256) | of_w0_k (384)]
    OW_SPLIT = [8, 4, 4]
    OW_START = [0, 8, 12]
    ow_d = [
        nc.declare_dram_parameter(f"ow{h}", [128, OW_SPLIT[h], 640], F8, isOutput=False)
        for h in range(3)
    ]
    # of window 1, k-split 8/4/4 so the final DMA chunk (and the matmul work
    # gated on it) is as small as possible
    OW1_SPLIT = [8, 4, 4]
    OW1_START = [0, 8, 12]
    ow1_d = [
        nc.declare_dram_parameter(f"ow1{h}", [128, OW1_SPLIT[h], W0], F8, isOutput=False)
        for h in range(3)
    ]
    # wm plus the four bias vectors as two extra bf16 columns per row
    wm_d = nc.declare_dram_parameter("wm", [128, 2, 514], BF, isOutput=False)
    # output in three pieces so each can DMA out as soon as it completes:
    # op = person half [c2,c3], ob0 = obj window-0 frames, ob1 = obj window-1
    out_p_d = nc.declare_dram_parameter("out_p", [128, 2, TP], BF, isOutput=True)
    out_b0_d = nc.declare_dram_parameter("out_b0", [128, 2, F0 * NP], BF, isOutput=True)
    out_b1_d = nc.declare_dram_parameter("out_b1", [128, 2, TP - F0 * NP], BF, isOutput=True)

    with tile.TileContext(nc) as tc:
        with (
            tc.tile_pool(name="loads", bufs=1) as loads,
            tc.tile_pool(name="work", bufs=1) as work,
            tc.tile_pool(name="psum", bufs=8, space="PSUM") as psum,
        ):
            # ---- input DMAs on the SP queue, in PE consumption order.  The
            #      fp8 wab rides as ONE transfer after a1 — many small
            #      transfers (sub-512B per partition line) starve the stream
            #      on issue cost and RMW penalties. ----
            a_sb = []
            wab_sb = None
            for i in range(5):
                t = loads.tile([128, A_SPLIT[i], 512], BF, tag=f"a{i}", name=f"a{i}")
                nc.sync.dma_start(t, a_d[i][:, :, :])
                a_sb.append(t)
                if i == 1:
                    wab_sb = loads.tile([128, KD, 256], F8, tag="wab", name="wab")
                    nc.sync.dma_start(wab_sb, wab_d[:, :, :])
            wm_sb = loads.tile([128, 2, 514], BF, tag="wm", name="wm")
            nc.sync.dma_start(wm_sb, wm_d[:, :, :])
            ow_sb = [None] * 3
            ow1_sb = [None] * 3
            for h in range(3):
                t = loads.tile([128, OW_SPLIT[h], 640], F8, tag=f"ow{h}", name=f"ow{h}")
                nc.sync.dma_start(t, ow_d[h][:, :, :])
                ow_sb[h] = t
            for h in range(3):
                t = loads.tile([128, OW1_SPLIT[h], W0], F8, tag=f"ow1{h}", name=f"ow1{h}")
                nc.sync.dma_start(t, ow1_d[h][:, :, :])
                ow1_sb[h] = t

            def achunk(k):
                for i in range(4, -1, -1):
                    if k >= A_START[i]:
                        return a_sb[i], k - A_START[i]
                raise AssertionError

            def wpchunk(k, m):  # Wp chunk (feeds P)
                t, kk = achunk(k)
                return t[:, kk, m * 128 : m * 128 + 128]

            def wabchunk(k, m):  # Wab chunk (fp8 stationary, feeds BP)
                return wab_sb[:, k, m * 128 : m * 128 + 128]

            def pfchunk(k):
                t, kk = achunk(k)
                return t[:, kk, 256:512]

            def wmchunk(kc, sec, m2):  # sec 0 = a_o (Wm1a), 1 = a_p (Wm2a)
                j0 = sec * 256 + m2 * 128
                return wm_sb[:, kc, j0 : j0 + 128]

            def _owc(g, kk):
                k = g * 4 + kk
                c = 0 if k < 8 else (1 if k < 12 else 2)
                return ow_sb[c], k - OW_START[c]

            def wobpair(g, kk, m2):  # [128, 2, 128] fp8 stationary, k-pair
                t, r = _owc(g, kk)
                return t[:, r : r + 2, m2 * 128 : m2 * 128 + 128]

            def ow0pair(g, kk):
                t, r = _owc(g, kk)
                return t[:, r : r + 2, 256:640]

            def ow1pair(g, kk, lo):
                k = g * 4 + kk
                c = 0 if k < 8 else (1 if k < 12 else 2)
                r = k - OW1_START[c]
                return ow1_sb[c][:, r : r + 2, lo : lo + 192]

            # bias views packed into wm: row 0 = object halves, row 1 = person
            def bias_obj(m2):  # [128, 1]
                return wm_sb[:, 0, 512 + m2 : 513 + m2]

            bias_per = wm_sb[:, 1, 512:514]  # [128, 2]

            # ---- PSUM: exactly 8 banks ----
            P_ps = psum.tile([128, 2, TP], F32, tag="ps", name="P_ps")
            BP_ps = psum.tile([128, 2, TP], F32, tag="ps", name="BP_ps")
            AP_ps = psum.tile([128, 2, TP], F32, tag="ps", name="AP_ps")
            AO_ps = psum.tile([128, 2, TP], F32, tag="ps", name="AO_ps")
            OB0 = [psum.tile([128, W0], F32, tag="ps", name=f"OB0_{m2}") for m2 in range(2)]
            OB1a = psum.tile([128, 2, 192], F32, tag="ps", name="OB1a")
            OB1b = psum.tile([128, 2, 192], F32, tag="ps", name="OB1b")

            # ---- PE warmup: the HAM clock gate needs ~3.4us of CONTINUOUS
            #      matmul activity to unthrottle 1.2 -> 2.4GHz, so junk
            #      matmuls fill the pre-data window AND every DMA-wait gap
            #      between phase-A chunks.  Mid-phase junk lands in the OB1b
            #      bank, which has no live accumulation until OB window 1
            #      (whose first real matmul has start=True and clears it). ----
            junk = work.tile([128, 256], BF, tag="junk", name="junk")
            nc.gpsimd.memset(junk, 0)

            def junk_mm(n, tgt=None, ncols=256):
                for _ in range(n):
                    nc.tensor.matmul(
                        tgt if tgt is not None else P_ps[:, 0, :],
                        junk[:, 0:128], junk[:, 0:ncols],
                        start=True, stop=True, skip_group_check=True,
                    )

            junk_mm(N_WARMUP)

            # ---- phase A, paced by a-chunk arrival.  Per chunk, P matmuls
            #      run first (their weights ride the a-chunks); BP trails by
            #      two chunks so the single wab transfer has landed.  Junk
            #      bursts bridge the pre-warm DMA gaps only — once the HAM
            #      clock is warm, short idles cost nothing.  The last chunk
            #      runs BP-first, m-half-major, so the PSUM->SBUF copies (and
            #      everything gated on them) pipeline with the remaining
            #      matmuls.  Mid-phase junk lands in OB1b's bank (no live
            #      accumulation there until OB window 1, whose first matmul
            #      clears it). ----
            def p_mms(ci, order):
                ks = range(A_START[ci], A_START[ci] + A_SPLIT[ci])
                for m, k in order(ks):
                    nc.tensor.matmul(P_ps[:, m, :], wpchunk(k, m), pfchunk(k),
                                     start=(k == 0 and m == 0),
                                     stop=(k == KD - 1))

            def bp_mms(ci, order):
                ks = range(A_START[ci], A_START[ci] + A_SPLIT[ci])
                for m, k in order(ks):
                    nc.tensor.matmul(BP_ps[:, m, :], wabchunk(k, m), pfchunk(k),
                                     start=(k == 0 and m == 0),
                                     stop=(k == KD - 1))

            k_major = lambda ks: [(m, k) for k in ks for m in range(2)]
            m_major = lambda ks: [(m, k) for m in range(2) for k in ks]
            p_mms(0, k_major)
            junk_mm(4, tgt=OB1b[:, 0, :], ncols=192)
            p_mms(1, k_major)
            junk_mm(3, tgt=OB1b[:, 0, :], ncols=192)
            bp_mms(0, k_major)
            bp_mms(1, k_major)
            for ci in (2, 3):
                p_mms(ci, k_major)
                bp_mms(ci, k_major)
            bp_mms(4, m_major)
            p_mms(4, m_major)

            # BP/P PSUM -> SBUF per m-half on Scalar, in retirement order, so
            # the person epilogue and stage 2 start as early as possible.  The
            # BP copy folds the 1/WAB_SCALE descale.
            CP = mybir.ActivationFunctionType.Copy
            BPc = work.tile([128, 2, TP], BF, tag="BPc", name="BPc")
            PT = work.tile([128, 2, TP], BF, tag="PTsb", name="PTsb")
            for m in range(2):
                nc.scalar.activation(BPc[:, m, :], BP_ps[:, m, :], CP,
                                     scale=1.0 / WAB_SCALE)
            for m in range(2):
                nc.scalar.copy(PT[:, m, :], P_ps[:, m, :])

            # ---- stage 2: a_p (AP) and a_o (AO) from PT, kc-outer so the
            #      kc=0 matmuls only wait on PT's first half ----
            for kc in range(2):
                for m2 in range(2):
                    st, sp = (kc == 0 and m2 == 0), (kc == 1)
                    nc.tensor.matmul(AP_ps[:, m2, :], wmchunk(kc, 1, m2), PT[:, kc, :], start=st, stop=sp)
                    nc.tensor.matmul(AO_ps[:, m2, :], wmchunk(kc, 0, m2), PT[:, kc, :], start=st, stop=sp)

            # ---- OB window 0: fp8 DoubleRow, two k-planes per matmul ----
            for g in range(4):
                for kk in (0, 2):
                    k = g * 4 + kk
                    for m2 in range(2):
                        nc.tensor.matmul(
                            OB0[m2], wobpair(g, kk, m2),
                            ow0pair(g, kk),
                            start=(k == 0), stop=(k == KD - 2),
                            perf_mode=DR,
                        )

            # ---- remaining PSUM -> SBUF copies (Scalar) ----
            APc = work.tile([128, 2, TP], BF, tag="APc", name="APc")
            nc.scalar.copy(APc, AP_ps)
            AOc = work.tile([128, 2, TP], BF, tag="AOc", name="AOc")
            nc.scalar.copy(AOc, AO_ps)

            # ---- persons epilogue (self-excluded max), bf16 on DVE
            # (free-axis reductions are DVE-only on TRN2; bf16 doubles DVE
            # throughput) ----
            SH3, SH4 = (128, 2, F_LOC), (128, 2, F_LOC, NP)
            out_p = work.tile([128, 2, TP], BF, tag="out_p", name="out_p")
            out_b0 = work.tile([128, 2, F0 * NP], BF, tag="out_b0", name="out_b0")
            out_b1 = work.tile([128, 2, TP - F0 * NP], BF, tag="out_b1", name="out_b1")
            V = nc.vector
            G = nc.gpsimd
            # high_priority pins this chain early in the static per-engine
            # order: without it the Tile scheduler slots the (matmul-gated)
            # object reduces ahead of m2v in the DVE FIFO, and the Pool mex
            # chain then starts ~2us late (head-of-line blocking).
            with tc.high_priority():
                bp4 = BPc.rearrange("p c (f i) -> p c f i", i=NP)
                m1 = work.tile(list(SH3), BF, tag="m1", name="m1")
                V.reduce_max(m1, bp4, axis=mybir.AxisListType.X)
                eq = work.tile(list(SH4), BF, tag="eq", name="eq")
                V.tensor_tensor(eq, bp4, m1[:, :, :, None].to_broadcast(SH4),
                                mybir.AluOpType.is_equal)
                msk = work.tile(list(SH4), BF, tag="msk", name="msk")
                V.scalar_tensor_tensor(msk, eq, -1e30, bp4,
                                       mybir.AluOpType.mult, mybir.AluOpType.add)
                m2v = work.tile(list(SH3), BF, tag="m2v", name="m2v")
                V.reduce_max(m2v, msk, axis=mybir.AxisListType.X)
                # the whole mex chain runs on Pool: ~2x slower per element,
                # but fully hidden behind the OB matmuls, and it keeps the
                # DVE FIFO free for the object reduces + adds
                dd = work.tile(list(SH3), BF, tag="dd", name="dd")
                G.tensor_tensor(dd, m2v, m1, mybir.AluOpType.subtract)
                m1pb = work.tile(list(SH3), BF, tag="m1pb", name="m1pb")
                G.tensor_tensor(m1pb, m1, bias_per[:, :, None].to_broadcast(SH3),
                                mybir.AluOpType.add)
                mex = work.tile(list(SH4), BF, tag="mex", name="mex")
                G.tensor_tensor(mex, eq, dd[:, :, :, None].to_broadcast(SH4),
                                mybir.AluOpType.mult)
                G.tensor_tensor(mex, mex, m1pb[:, :, :, None].to_broadcast(SH4),
                                mybir.AluOpType.add)
                G.tensor_tensor(
                    out_p.rearrange("p c (f i) -> p c f i", i=NP),
                    APc.rearrange("p c (f i) -> p c f i", i=NP),
                    mex, mybir.AluOpType.add,
                )
            nc.scalar.dma_start(out_p_d[:, :, :], out_p)

            # ---- OB window 1: k-pair outer, both half-windows inner, so each
            #      incoming ow1 chunk is fully consumed before the next is
            #      needed and the post-last-chunk matmul tail is 8 MMs.  The
            #      a-half's stop lands before the b-half's, so its reduce
            #      overlaps the b-half's final matmuls. ----
            for g in range(4):
                for kk in (0, 2):
                    k = g * 4 + kk
                    for OB1, lo in ((OB1a, 0), (OB1b, 192)):
                        for m2 in range(2):
                            nc.tensor.matmul(
                                OB1[:, m2, :], wobpair(g, kk, m2),
                                ow1pair(g, kk, lo),
                                start=(k == 0 and m2 == 0),
                                stop=(k == KD - 2),
                                perf_mode=DR,
                            )

            # ---- object epilogues ----
            # Reduces straight from PSUM on DVE (the only engine that can).
            # Window 1 is the critical output chain: its descale + fused
            # scalar_tensor_tensor adds stay on DVE right behind the reduces.
            # Window 0 is NOT critical (its data is ready ~4us before the
            # last matmul), so its fixups run decomposed on Pool after the
            # mex chain — off the DVE FIFO entirely.
            maxo = work.tile([128, 2, 2, F0], F32, tag="maxo", name="maxo")
            maxo0 = work.tile([128, 2, F0], BF, tag="maxo0", name="maxo0")
            bias2 = wm_sb[:, 0, 512:514]  # [128, 2] object bias halves

            def obj_add(m2, hslc, t0, nfr, dst, d0):
                V.scalar_tensor_tensor(
                    dst[:, m2, d0 : d0 + nfr * NP].rearrange(
                        "p (f i) -> p f i", i=NP
                    ),
                    maxo[:, 1, m2, hslc, None].to_broadcast((128, nfr, NP)),
                    bias_obj(m2),
                    AOc[:, m2, t0 : t0 + nfr * NP].rearrange("p (f i) -> p f i", i=NP),
                    mybir.AluOpType.add,
                    mybir.AluOpType.add,
                )

            # window 0 (frames 0-7): DVE reduces (bf16 out), Pool fixups
            for m2 in range(2):
                V.reduce_max(
                    maxo0[:, m2, :],
                    OB0[m2].rearrange("p (f o) -> p f o", o=NO),
                    axis=mybir.AxisListType.X,
                )
            G.tensor_scalar_mul(maxo0, maxo0, 1.0 / WOB_SCALE)
            G.tensor_tensor(maxo0, maxo0, bias2[:, :, None].to_broadcast((128, 2, F0)),
                            mybir.AluOpType.add)
            G.tensor_tensor(
                out_b0.rearrange("p c (f i) -> p c f i", i=NP),
                maxo0[:, :, :, None].to_broadcast((128, 2, F0, NP)),
                AOc[:, :, 0 : F0 * NP].rearrange("p c (f i) -> p c f i", i=NP),
                mybir.AluOpType.add,
            )
            nc.scalar.dma_start(out_b0_d[:, :, :], out_b0)

            # window 1, half A (frames 8-11) then half B (12-15) — all DVE
            for h, OB1 in ((0, OB1a), (1, OB1b)):
                hs = slice(4 * h, 4 * h + 4)
                V.reduce_max(
                    maxo[:, 1, :, hs],
                    OB1.rearrange("p c (f o) -> p c f o", o=NO),
                    axis=mybir.AxisListType.X,
                )
                V.tensor_scalar_mul(maxo[:, 1, :, hs], maxo[:, 1, :, hs], 1.0 / WOB_SCALE)
                for m2 in range(2):
                    obj_add(m2, hs, 128 + 64 * h, 4, out_b1, 64 * h)
            nc.scalar.dma_start(out_b1_d[:, :, :], out_b1)

    nc.compile()
    return nc


def _get_nc():
    global _NC_CACHE
    if _NC_CACHE is None:
        _NC_CACHE = _build_nc()
    return _NC_CACHE


def _marshal(pf, of, Wp, bp, Wpr, bpr, Wo, bo, Wm_obj, bm_obj, Wm_per, bm_per):
    """Pack full f32 inputs into per-core DRAM parameter layouts."""
    pf_bf = pf.astype(BF16)
    of_q = of.astype(FP8)

    Wab = Wpr @ Wm_per[C:]                                               # [D, C] fused BP weight
    Wob = Wo @ Wm_obj[C:]                                                # [D, C] fused OB weight
    wp_packed = Wp.astype(BF16).reshape(KD, 128, 256).transpose(1, 0, 2)  # [128, KD, 256]
    wab_packed = (Wab * WAB_SCALE).astype(FP8).reshape(KD, 128, 256).transpose(1, 0, 2)
    wob_packed = (Wob * WOB_SCALE).astype(FP8).reshape(KD, 128, 256).transpose(1, 0, 2)
    wmcat = np.concatenate([Wm_obj[:C], Wm_per[:C]], axis=1).astype(BF16)  # [C, 512]
    wm_packed = wmcat.reshape(2, 128, 512).transpose(1, 0, 2)            # [128, 2, 512]

    bias_obj = bm_obj + bp @ Wm_obj[:C] + bo @ Wm_obj[C:]
    bias_per = bm_per + bp @ Wm_per[:C] + bpr @ Wm_per[C:]
    # bias rides in wm as two extra bf16 columns: row 0 obj halves, row 1 per
    bias4 = np.stack(
        [bias_obj[0:128], bias_obj[128:256], bias_per[0:128], bias_per[128:256]],
        axis=1,
    ).astype(BF16)                                                       # [128, 4]
    wmb = np.concatenate([wm_packed, bias4.reshape(128, 2, 2)], axis=2)  # [128, 2, 514]
    wmb = np.ascontiguousarray(wmb)

    in_maps = []
    for c in range(NCORES):
        pfc = pf_bf[c * TP : (c + 1) * TP]                                # [TP, D]
        ofc = of_q[c * TO : (c + 1) * TO]                                 # [TO, D]
        pf_packed = pfc.reshape(TP, KD, 128).transpose(2, 1, 0)           # [128, KD, TP]
        a_full = np.concatenate([wp_packed, pf_packed], axis=2)           # [128, KD, 512]
        of_packed = ofc.reshape(TO, KD, 128).transpose(2, 1, 0)           # [128, KD, TO]
        owcat = np.concatenate(
            [wob_packed, of_packed[:, :, 0:W0]], axis=2
        )                                                                 # [128, KD, 640]
        m = {"wm": wmb, "wab8": wab_packed}
        for i in range(5):
            m[f"a{i}"] = np.ascontiguousarray(
                a_full[:, A_START[i] : A_START[i] + A_SPLIT[i], :]
            )
        for h, (k0, kn) in enumerate(((0, 8), (8, 4), (12, 4))):
            m[f"ow{h}"] = np.ascontiguousarray(owcat[:, k0 : k0 + kn, :])
            m[f"ow1{h}"] = np.ascontiguousarray(
                of_packed[:, k0 : k0 + kn, W0:TO]
            )
        in_maps.append(m)
    return in_maps


def _unmarshal(results):
    """Per-core 3-piece bf16 output -> [F*NP, 2C, 1,1,1] f32."""
    blocks = []
    for c in range(NCORES):
        r = results[c]
        obj = np.concatenate(
            [np.asarray(r["out_b0"]), np.asarray(r["out_b1"])], axis=2
        ).astype(np.float32)                                              # [128, 2, TP]
        per = np.asarray(r["out_p"]).astype(np.float32)                   # [128, 2, TP]
        arr = np.concatenate([obj, per], axis=1)                          # [128, 4, TP]
        out_t = arr.transpose(1, 0, 2).reshape(2 * C, TP)                 # [512, TP]
        blocks.append(out_t.T)                                            # [TP, 512]
    full = np.concatenate(blocks, axis=0).astype(np.float32)              # [F*NP, 2C]
    return full[:, :, None, None, None]


def kernel(
    person_feature,
    obj_feature,
    Wp,
    bp,
    Wpr,
    bpr,
    Wo,
    bo,
    Wm_obj,
    bm_obj,
    Wm_per,
    bm_per,
    f_num,
    np_pf,
    no_pf,
):
    assert int(f_num) == F and int(np_pf) == NP and int(no_pf) == NO
    pf = np.asarray(person_feature, dtype=np.float32)[:, :, 0, 0, 0]
    of = np.asarray(obj_feature, dtype=np.float32)[:, :, 0, 0, 0]
    args = [
        np.asarray(a, dtype=np.float32)
        for a in (Wp, bp, Wpr, bpr, Wo, bo, Wm_obj, bm_obj, Wm_per, bm_per)
    ]
    in_maps = _marshal(pf, of, *args)
    nc = _get_nc()
    res = run_bass_kernel_spmd(nc, in_maps, core_ids=list(range(NCORES)))
    return _unmarshal(res.results)


if __name__ == "__main__":
    # smoke test with random data against a numpy re-derivation
    rng = np.random.default_rng(0)
    pf = rng.standard_normal((F * NP, D, 1, 1, 1), dtype=np.float32)
    of = rng.standard_normal((F * NO, D, 1, 1, 1), dtype=np.float32)
    mk = lambda *s: (rng.standard_normal(s, dtype=np.float32) * 0.01)
    inputs = dict(
        person_feature=pf,
        obj_feature=of,
        Wp=mk(D, C),
        bp=np.zeros(C, np.float32),
        Wpr=mk(D, C),
        bpr=np.zeros(C, np.float32),
        Wo=mk(D, C),
        bo=np.zeros(C, np.float32),
        Wm_obj=rng.standard_normal((2 * C, C), dtype=np.float32) / np.sqrt(2 * C),
        bm_obj=np.zeros(C, np.float32),
        Wm_per=rng.standard_normal((2 * C, C), dtype=np.float32) / np.sqrt(2 * C),
        bm_per=np.zeros(C, np.float32),
        f_num=F,
        np_pf=NP,
        no_pf=NO,
    )
    out = kernel(**inputs)
    print("kernel output shape:", out.shape)


# revision 48
# speedup vs baseline: 1.0935x; 1.0140x over previous
"""Trainium2 Bass kernel for nn_AOGStructure (gnn_message_passing).

Reference computation (per frame f, with NP persons / NO objects, C=256):
    P = pf @ Wp + bp            # persons_red
    A = pf @ Wpr + bpr          # act_persons_red
    O = of @ Wo + bo            # objs_red
    objs_interact[f,i]    = max_j       (P[f,i] @ Wm_obj[:C] + O[f,j] @ Wm_obj[C:] + bm_obj)
    persons_interact[f,i] = max_{j!=i}  (P[f,i] @ Wm_per[:C] + A[f,j] @ Wm_per[C:] + bm_per)
    out = concat([objs_interact, persons_interact], -1)

Since the per-pair message is additive in (i-term, j-term), the max over j
factorizes:  max_j (a_i + b_j) = a_i + max_j b_j.  The [F,NP,NO,C] pair tensor
is never materialized.  For the person block the self-excluded max is computed
from the max and the masked ("second") max.  All biases commute with the max
and are folded into a single per-output-channel bias vector added at the end.

Strategy: data-parallel over frames, 16 frames per core, weights replicated,
no collectives.  A single DMA stream whose transfer order equals PE
consumption order, issued as ~15 large contiguous DMAs (per-DMA issue costs
~0.65us on the SP queue, so small transfers are ruinous):

  phase A   5 chunks of [wpa_k | pf_k]   -> BP/P matmuls   (bf16)
  wm/bias                                -> stage 2 + epilogues
  phase B   4x [wob_g | of_w0_g]         -> OB window-0    (fp8 DoubleRow)
  phase C   4x of_w1_g                   -> OB window-1 (two half-windows)

The whole object path runs in fp8-e4m3: `of` quantized directly, Wob
pre-scaled by 2048 (73% of Wob underflows into e4m3 subnormals unscaled) and
the 1/2048 folded into the per-window max fixup.  Both operands fp8 enables
MatmulPerfMode.DoubleRow: two contraction rows per PE cycle, halving the OB
phase.

The PE HAM clock gate defaults to 1.2GHz and only reaches 2.4GHz after
~3.4us of *continuous* matmul activity; any idle gap restarts the ramp.  So
junk matmuls (into the not-yet-live OB1b PSUM bank) fill every DMA-wait gap:
a burst before phase A and a small burst before each chunk-gated LDWEIGHTS.
Within phase A the BP matmuls run before P per chunk so BP retires first and
the person epilogue (the long DVE chain) starts as early as possible.

Epilogues are split across engines: DVE owns PSUM reads (reduce_max) and the
m2=0 half of the person chain; Pool (gpsimd) gets the m2=1 half plus all
SBUF-only object fixups.  PSUM->SBUF staging copies run on Scalar.  Output is
bf16 in three params (person half / obj window-0 / obj window-1), each DMA'd
on the Scalar queue as soon as its last add lands, so the final transfer is a
64KB sliver instead of the whole output.  PSUM uses exactly 8 banks.
"""

import sys

if "/opt/trn_rl_repo" not in sys.path:
    sys.path.insert(0, "/opt/trn_rl_repo")

import ml_dtypes
import numpy as np

import concourse.bass as bass  # noqa: F401  (import keeps bass registered)
import concourse.tile as tile
from concourse import bacc, mybir
from concourse.bass_utils import run_bass_kernel_spmd

NCORES = 8
F, NP, NO = 128, 16, 48
D, C = 2048, 256
F_LOC = F // NCORES          # 16 frames per core
TP = F_LOC * NP              # 256 person tokens per core
TO = F_LOC * NO              # 768 object tokens per core
KD = D // 128                # 16 contraction chunks of 128
W0 = 384                     # of window 0: frames 0-7
F0 = W0 // NO                # 8 frames in window 0
WOB_SCALE = 2048.0           # keeps fp8 Wob out of the subnormal range
BF16 = ml_dtypes.bfloat16
FP8 = ml_dtypes.float8_e4m3

# k-extents of the five phase-A chunks (first small so the PE starts early)
A_SPLIT = [1, 3, 4, 4, 4]
A_START = [0, 1, 4, 8, 12]
N_WARMUP = 12                # junk matmuls to ramp the PE clock before data
WAB_SCALE = 128.0            # keeps fp8 Wab out of the subnormal range

_NC_CACHE = None


def _build_nc():
    """Build the single-core SPMD graph (same NEFF on all 8 cores)."""
    nc = bacc.Bacc("TRN2", target_bir_lowering=False, debug=False)
    BF = mybir.dt.bfloat16
    F8 = mybir.dt.float8e4
    F32 = mybir.dt.float32
    DR = mybir.MatmulPerfMode.DoubleRow

    a_d = [
        nc.declare_dram_parameter(f"a{i}", [128, A_SPLIT[i], 512], BF, isOutput=False)
        for i in range(5)
    ]
    # fused BP weight ships as fp8 (x128 to clear the subnormal range) and is
    # consumed DIRECTLY as the fp8 stationary operand against bf16 pf (the PE
    # allows mixed non-fp32 operand dtypes; HW-verified exact) — half the HBM
    # bytes of a bf16 Wab at identical matmul cost
    wab_d = nc.declare_dram_parameter("wab8", [128, KD, 256], F8, isOutput=False)
    # merged fp8 chunks, k-split 8/4/4: per k-row [wob_k (256) | of_w0_k (384)]
    OW_SPLIT = [8, 4, 4]
    OW_START = [0, 8, 12]
    ow_d = [
        nc.declare_dram_parameter(f"ow{h}", [128, OW_SPLIT[h], 640], F8, isOutput=False)
        for h in range(3)
    ]
    # of window 1, k-split 8/4/4 so the final DMA chunk (and the matmul work
    # gated on it) is as small as possible
    OW1_SPLIT = [8, 4, 4]
    OW1_START = [0, 8, 12]
    ow1_d = [
        nc.declare_dram_parameter(f"ow1{h}", [128, OW1_SPLIT[h], W0], F8, isOutput=False)
        for h in range(3)
    ]
    # wm plus the four bias vectors as two extra bf16 columns per row
    wm_d = nc.declare_dram_parameter("wm", [128, 2, 514], BF, isOutput=False)
    # output in three pieces so each can DMA out as soon as it completes:
    # op = person half [c2,c3], ob0 = obj window-0 frames, ob1 = obj window-1
    out_p_d = nc.declare_dram_parameter("out_p", [128, 2, TP], BF, isOutput=True)
    out_b0_d = nc.declare_dram_parameter("out_b0", [128, 2, F0 * NP], BF, isOutput=True)
    out_b1_d = nc.declare_dram_parameter("out_b1", [128, 2, TP - F0 * NP], BF, isOutput=True)

    with tile.TileContext(nc) as tc:
        with (
            tc.tile_pool(name="loads", bufs=1) as loads,
            tc.tile_pool(name="work", bufs=1) as work,
            tc.tile_pool(name="psum", bufs=8, space="PSUM") as psum,
        ):
            # ---- input DMAs on the SP queue, in PE consumption order.  The
            #      fp8 wab rides as ONE transfer after a1 — many small
            #      transfers (sub-512B per partition line) starve the stream
            #      on issue cost and RMW penalties. ----
            a_sb = []
            wab_sb = None
            for i in range(5):
                t = loads.tile([128, A_SPLIT[i], 512], BF, tag=f"a{i}", name=f"a{i}")
                nc.sync.dma_start(t, a_d[i][:, :, :])
                a_sb.append(t)
                if i == 1:
                    wab_sb = loads.tile([128, KD, 256], F8, tag="wab", name="wab")
                    nc.sync.dma_start(wab_sb, wab_d[:, :, :])
            wm_sb = loads.tile([128, 2, 514], BF, tag="wm", name="wm")
            nc.sync.dma_start(wm_sb, wm_d[:, :, :])
            ow_sb = [None] * 3
            ow1_sb = [None] * 3
            for h in range(3):
                t = loads.tile([128, OW_SPLIT[h], 640], F8, tag=f"ow{h}", name=f"ow{h}")
                nc.sync.dma_start(t, ow_d[h][:, :, :])
                ow_sb[h] = t
            for h in range(3):
                t = loads.tile([128, OW1_SPLIT[h], W0], F8, tag=f"ow1{h}", name=f"ow1{h}")
                nc.sync.dma_start(t, ow1_d[h][:, :, :])
                ow1_sb[h] = t

            def achunk(k):
                for i in range(4, -1, -1):
                    if k >= A_START[i]:
                        return a_sb[i], k - A_START[i]
                raise AssertionError

            def wpchunk(k, m):  # Wp chunk (feeds P)
                t, kk = achunk(k)
                return t[:, kk, m * 128 : m * 128 + 128]

            def wabchunk(k, m):  # Wab chunk (fp8 stationary, feeds BP)
                return wab_sb[:, k, m * 128 : m * 128 + 128]

            def pfchunk(k):
                t, kk = achunk(k)
                return t[:, kk, 256:512]

            def wmchunk(kc, sec, m2):  # sec 0 = a_o (Wm1a), 1 = a_p (Wm2a)
                j0 = sec * 256 + m2 * 128
                return wm_sb[:, kc, j0 : j0 + 128]

            def _owc(g, kk):
                k = g * 4 + kk
                c = 0 if k < 8 else (1 if k < 12 else 2)
                return ow_sb[c], k - OW_START[c]

            def wobpair(g, kk, m2):  # [128, 2, 128] fp8 stationary, k-pair
                t, r = _owc(g, kk)
                return t[:, r : r + 2, m2 * 128 : m2 * 128 + 128]

            def ow0pair(g, kk):
                t, r = _owc(g, kk)
                return t[:, r : r + 2, 256:640]

            def ow1pair(g, kk, lo):
                k = g * 4 + kk
                c = 0 if k < 8 else (1 if k < 12 else 2)
                r = k - OW1_START[c]
                return ow1_sb[c][:, r : r + 2, lo : lo + 192]

            # bias views packed into wm: row 0 = object halves, row 1 = person
            def bias_obj(m2):  # [128, 1]
                return wm_sb[:, 0, 512 + m2 : 513 + m2]

            bias_per = wm_sb[:, 1, 512:514]  # [128, 2]

            # ---- PSUM: exactly 8 banks ----
            P_ps = psum.tile([128, 2, TP], F32, tag="ps", name="P_ps")
            BP_ps = psum.tile([128, 2, TP], F32, tag="ps", name="BP_ps")
            AP_ps = psum.tile([128, 2, TP], F32, tag="ps", name="AP_ps")
            AO_ps = psum.tile([128, 2, TP], F32, tag="ps", name="AO_ps")
            OB0 = [psum.tile([128, W0], F32, tag="ps", name=f"OB0_{m2}") for m2 in range(2)]
            OB1a = psum.tile([128, 2, 192], F32, tag="ps", name="OB1a")
            OB1b = psum.tile([128, 2, 192], F32, tag="ps", name="OB1b")

            # ---- PE warmup: the HAM clock gate needs ~3.4us of CONTINUOUS
            #      matmul activity to unthrottle 1.2 -> 2.4GHz, so junk
            #      matmuls fill the pre-data window AND every DMA-wait gap
            #      between phase-A chunks.  Mid-phase junk lands in the OB1b
            #      bank, which has no live accumulation until OB window 1
            #      (whose first real matmul has start=True and clears it). ----
            junk = work.tile([128, 256], BF, tag="junk", name="junk")
            nc.gpsimd.memset(junk, 0)

            def junk_mm(n, tgt=None, ncols=256):
                for _ in range(n):
                    nc.tensor.matmul(
                        tgt if tgt is not None else P_ps[:, 0, :],
                        junk[:, 0:128], junk[:, 0:ncols],
                        start=True, stop=True, skip_group_check=True,
                    )

            junk_mm(N_WARMUP)

            # ---- phase A, paced by a-chunk arrival.  Per chunk, P matmuls
            #      run first (their weights ride the a-chunks); BP trails by
            #      two chunks so the single wab transfer has landed.  Junk
            #      bursts bridge the pre-warm DMA gaps only — once the HAM
            #      clock is warm, short idles cost nothing.  The last chunk
            #      runs BP-first, m-half-major, so the PSUM->SBUF copies (and
            #      everything gated on them) pipeline with the remaining
            #      matmuls.  Mid-phase junk lands in OB1b's bank (no live
            #      accumulation there until OB window 1, whose first matmul
            #      clears it). ----
            def p_mms(ci, order):
                ks = range(A_START[ci], A_START[ci] + A_SPLIT[ci])
                for m, k in order(ks):
                    nc.tensor.matmul(P_ps[:, m, :], wpchunk(k, m), pfchunk(k),
                                     start=(k == 0 and m == 0),
                                     stop=(k == KD - 1))

            def bp_mms(ci, order):
                ks = range(A_START[ci], A_START[ci] + A_SPLIT[ci])
                for m, k in order(ks):
                    nc.tensor.matmul(BP_ps[:, m, :], wabchunk(k, m), pfchunk(k),
                                     start=(k == 0 and m == 0),
                                     stop=(k == KD - 1))

            k_major = lambda ks: [(m, k) for k in ks for m in range(2)]
            m_major = lambda ks: [(m, k) for m in range(2) for k in ks]
            p_mms(0, k_major)
            junk_mm(4, tgt=OB1b[:, 0, :], ncols=192)
            p_mms(1, k_major)
            junk_mm(3, tgt=OB1b[:, 0, :], ncols=192)
            bp_mms(0, k_major)
            bp_mms(1, k_major)
            for ci in (2, 3):
                p_mms(ci, k_major)
                bp_mms(ci, k_major)
            bp_mms(4, m_major)
            p_mms(4, m_major)

            # BP/P PSUM -> SBUF per m-half on Scalar, in retirement order, so
            # the person epilogue and stage 2 start as early as possible.  The
            # BP copy folds the 1/WAB_SCALE descale.
            CP = mybir.ActivationFunctionType.Copy
            BPc = work.tile([128, 2, TP], BF, tag="BPc", name="BPc")
            PT = work.tile([128, 2, TP], BF, tag="PTsb", name="PTsb")
            for m in range(2):
                nc.scalar.activation(BPc[:, m, :], BP_ps[:, m, :], CP,
                                     scale=1.0 / WAB_SCALE)
            for m in range(2):
                nc.scalar.copy(PT[:, m, :], P_ps[:, m, :])

            # ---- stage 2: a_p (AP) and a_o (AO) from PT, kc-outer so the
            #      kc=0 matmuls only wait on PT's first half ----
            for kc in range(2):
                for m2 in range(2):
                    st, sp = (kc == 0 and m2 == 0), (kc == 1)
                    nc.tensor.matmul(AP_ps[:, m2, :], wmchunk(kc, 1, m2), PT[:, kc, :], start=st, stop=sp)
                    nc.tensor.matmul(AO_ps[:, m2, :], wmchunk(kc, 0, m2), PT[:, kc, :], start=st, stop=sp)

            # ---- OB window 0: fp8 DoubleRow, two k-planes per matmul ----
            for g in range(4):
                for kk in (0, 2):
                    k = g * 4 + kk
                    for m2 in range(2):
                        nc.tensor.matmul(
                            OB0[m2], wobpair(g, kk, m2),
                            ow0pair(g, kk),
                            start=(k == 0), stop=(k == KD - 2),
                            perf_mode=DR,
                        )

            # ---- remaining PSUM -> SBUF copies (Scalar) ----
            APc = work.tile([128, 2, TP], BF, tag="APc", name="APc")
            nc.scalar.copy(APc, AP_ps)
            AOc = work.tile([128, 2, TP], BF, tag="AOc", name="AOc")
            nc.scalar.copy(AOc, AO_ps)

            # ---- persons epilogue (self-excluded max), bf16 on DVE
            # (free-axis reductions are DVE-only on TRN2; bf16 doubles DVE
            # throughput) ----
            SH3, SH4 = (128, 2, F_LOC), (128, 2, F_LOC, NP)
            out_p = work.tile([128, 2, TP], BF, tag="out_p", name="out_p")
            out_b0 = work.tile([128, 2, F0 * NP], BF, tag="out_b0", name="out_b0")
            out_b1 = work.tile([128, 2, TP - F0 * NP], BF, tag="out_b1", name="out_b1")
            V = nc.vector
            G = nc.gpsimd
            # high_priority pins this chain early in the static per-engine
            # order: without it the Tile scheduler slots the (matmul-gated)
            # object reduces ahead of m2v in the DVE FIFO, and the Pool mex
            # chain then starts ~2us late (head-of-line blocking).
            with tc.high_priority():
                bp4 = BPc.rearrange("p c (f i) -> p c f i", i=NP)
                m1 = work.tile(list(SH3), BF, tag="m1", name="m1")
                V.reduce_max(m1, bp4, axis=mybir.AxisListType.X)
                eq = work.tile(list(SH4), BF, tag="eq", name="eq")
                V.tensor_tensor(eq, bp4, m1[:, :, :, None].to_broadcast(SH4),
                                mybir.AluOpType.is_equal)
                msk = work.tile(list(SH4), BF, tag="msk", name="msk")
                V.scalar_tensor_tensor(msk, eq, -1e30, bp4,
                                       mybir.AluOpType.mult, mybir.AluOpType.add)
                m2v = work.tile(list(SH3), BF, tag="m2v", name="m2v")
                V.reduce_max(m2v, msk, axis=mybir.AxisListType.X)
                # the whole mex chain runs on Pool: ~2x slower per element,
                # but fully hidden behind the OB matmuls, and it keeps the
                # DVE FIFO free for the object reduces + adds
                dd = work.tile(list(SH3), BF, tag="dd", name="dd")
                G.tensor_tensor(dd, m2v, m1, mybir.AluOpType.subtract)
                m1pb = work.tile(list(SH3), BF, tag="m1pb", name="m1pb")
                G.tensor_tensor(m1pb, m1, bias_per[:, :, None].to_broadcast(SH3),
                                mybir.AluOpType.add)
                mex = work.tile(list(SH4), BF, tag="mex", name="mex")
                G.tensor_tensor(mex, eq, dd[:, :, :, None].to_broadcast(SH4),
                                mybir.AluOpType.mult)
                G.tensor_tensor(mex, mex, m1pb[:, :, :, None].to_broadcast(SH4),
                                mybir.AluOpType.add)
                G.tensor_tensor(
                    out_p.rearrange("p c (f i) -> p c f i", i=NP),
                    APc.rearrange("p c (f i) -> p c f i", i=NP),
                    mex, mybir.AluOpType.add,
                )
            nc.scalar.dma_start(out_p_d[:, :, :], out_p)

            # ---- OB window 1: k-pair outer, both half-windows inner, so each
            #      incoming ow1 chunk is fully consumed before the next is
            #      needed and the post-last-chunk matmul tail is 8 MMs.  The
            #      a-half's stop lands before the b-half's, so its reduce
            #      overlaps the b-half's final matmuls. ----
            for g in range(4):
                for kk in (0, 2):
                    k = g * 4 + kk
                    for OB1, lo in ((OB1a, 0), (OB1b, 192)):
                        for m2 in range(2):
                            nc.tensor.matmul(
                                OB1[:, m2, :], wobpair(g, kk, m2),
                                ow1pair(g, kk, lo),
                                start=(k == 0 and m2 == 0),
                                stop=(k == KD - 2),
                                perf_mode=DR,
                            )

            # ---- object epilogues ----
            # Reduces straight from PSUM on DVE (the only engine that can).
            # Window 1 is the critical output chain: its descale + fused
            # scalar_tensor_tensor adds stay on DVE right behind the reduces.
            # Window 0 is NOT critical (its data is ready ~4us before the
            # last matmul), so its fixups run decomposed on Pool after the
            # mex chain — off the DVE FIFO entirely.
            maxo0 = work.tile([128, 2, F0], BF, tag="maxo0", name="maxo0")
            maxo1 = work.tile([128, 2, F0], BF, tag="maxo1", name="maxo1")
            bias2 = wm_sb[:, 0, 512:514]  # [128, 2] object bias halves

            # window 0 (frames 0-7): DVE reduces (bf16 out), Pool fixups
            for m2 in range(2):
                V.reduce_max(
                    maxo0[:, m2, :],
                    OB0[m2].rearrange("p (f o) -> p f o", o=NO),
                    axis=mybir.AxisListType.X,
                )
            G.tensor_scalar_mul(maxo0, maxo0, 1.0 / WOB_SCALE)
            G.tensor_tensor(maxo0, maxo0, bias2[:, :, None].to_broadcast((128, 2, F0)),
                            mybir.AluOpType.add)
            G.tensor_tensor(
                out_b0.rearrange("p c (f i) -> p c f i", i=NP),
                maxo0[:, :, :, None].to_broadcast((128, 2, F0, NP)),
                AOc[:, :, 0 : F0 * NP].rearrange("p c (f i) -> p c f i", i=NP),
                mybir.AluOpType.add,
            )
            nc.scalar.dma_start(out_b0_d[:, :, :], out_b0)

            # window 1, half A (frames 8-11) then half B (12-15) — all DVE,
            # all bf16, bias folded into the tiny maxo tile so each half is
            # one reduce + two tiny fixups + ONE broadcast add.  The critical
            # out_b1 DMA issues from the (long-idle) SP queue so it never
            # waits behind the other outputs on the Scalar FIFO.
            for h, OB1 in ((0, OB1a), (1, OB1b)):
                hs = slice(4 * h, 4 * h + 4)
                V.reduce_max(
                    maxo1[:, :, hs],
                    OB1.rearrange("p c (f o) -> p c f o", o=NO),
                    axis=mybir.AxisListType.X,
                )
                V.tensor_scalar_mul(maxo1[:, :, hs], maxo1[:, :, hs], 1.0 / WOB_SCALE)
                V.tensor_tensor(maxo1[:, :, hs],
                                maxo1[:, :, hs],
                                bias2[:, :, None].to_broadcast((128, 2, 4)),
                                mybir.AluOpType.add)
                V.tensor_tensor(
                    out_b1[:, :, 64 * h : 64 * h + 64].rearrange(
                        "p c (f i) -> p c f i", i=NP
                    ),
                    maxo1[:, :, hs, None].to_broadcast((128, 2, 4, NP)),
                    AOc[:, :, 128 + 64 * h : 192 + 64 * h].rearrange(
                        "p c (f i) -> p c f i", i=NP
                    ),
                    mybir.AluOpType.add,
                )
            nc.sync.dma_start(out_b1_d[:, :, :], out_b1)

    nc.compile()
    return nc


def _get_nc():
    global _NC_CACHE
    if _NC_CACHE is None:
        _NC_CACHE = _build_nc()
    return _NC_CACHE


def _marshal(pf, of, Wp, bp, Wpr, bpr, Wo, bo, Wm_obj, bm_obj, Wm_per, bm_per):
    """Pack full f32 inputs into per-core DRAM parameter layouts."""
    pf_bf = pf.astype(BF16)
    of_q = of.astype(FP8)

    Wab = Wpr @ Wm_per[C:]                                               # [D, C] fused BP weight
    Wob = Wo @ Wm_obj[C:]                                                # [D, C] fused OB weight
    wp_packed = Wp.astype(BF16).reshape(KD, 128, 256).transpose(1, 0, 2)  # [128, KD, 256]
    wab_packed = (Wab * WAB_SCALE).astype(FP8).reshape(KD, 128, 256).transpose(1, 0, 2)
    wob_packed = (Wob * WOB_SCALE).astype(FP8).reshape(KD, 128, 256).transpose(1, 0, 2)
    wmcat = np.concatenate([Wm_obj[:C], Wm_per[:C]], axis=1).astype(BF16)  # [C, 512]
    wm_packed = wmcat.reshape(2, 128, 512).transpose(1, 0, 2)            # [128, 2, 512]

    bias_obj = bm_obj + bp @ Wm_obj[:C] + bo @ Wm_obj[C:]
    bias_per = bm_per + bp @ Wm_per[:C] + bpr @ Wm_per[C:]
    # bias rides in wm as two extra bf16 columns: row 0 obj halves, row 1 per
    bias4 = np.stack(
        [bias_obj[0:128], bias_obj[128:256], bias_per[0:128], bias_per[128:256]],
        axis=1,
    ).astype(BF16)                                                       # [128, 4]
    wmb = np.concatenate([wm_packed, bias4.reshape(128, 2, 2)], axis=2)  # [128, 2, 514]
    wmb = np.ascontiguousarray(wmb)

    in_maps = []
    for c in range(NCORES):
        pfc = pf_bf[c * TP : (c + 1) * TP]                                # [TP, D]
        ofc = of_q[c * TO : (c + 1) * TO]                                 # [TO, D]
        pf_packed = pfc.reshape(TP, KD, 128).transpose(2, 1, 0)           # [128, KD, TP]
        a_full = np.concatenate([wp_packed, pf_packed], axis=2)           # [128, KD, 512]
        of_packed = ofc.reshape(TO, KD, 128).transpose(2, 1, 0)           # [128, KD, TO]
        owcat = np.concatenate(
            [wob_packed, of_packed[:, :, 0:W0]], axis=2
        )                                                                 # [128, KD, 640]
        m = {"wm": wmb, "wab8": wab_packed}
        for i in range(5):
            m[f"a{i}"] = np.ascontiguousarray(
                a_full[:, A_START[i] : A_START[i] + A_SPLIT[i], :]
            )
        for h, (k0, kn) in enumerate(((0, 8), (8, 4), (12, 4))):
            m[f"ow{h}"] = np.ascontiguousarray(owcat[:, k0 : k0 + kn, :])
            m[f"ow1{h}"] = np.ascontiguousarray(
                of_packed[:, k0 : k0 + kn, W0:TO]
            )
        in_maps.append(m)
    return in_maps


def _unmarshal(results):
    """Per-core 3-piece bf16 output -> [F*NP, 2C, 1,1,1] f32."""
    blocks = []
    for c in range(NCORES):
        r = results[c]
        obj = np.concatenate(
            [np.asarray(r["out_b0"]), np.asarray(r["out_b1"])], axis=2
        ).astype(np.float32)                                              # [128, 2, TP]
        per = np.asarray(r["out_p"]).astype(np.float32)                   # [128, 2, TP]
        arr = np.concatenate([obj, per], axis=1)                          # [128, 4, TP]
        out_t = arr.transpose(1, 0, 2).reshape(2 * C, TP)                 # [512, TP]
        blocks.append(out_t.T)                                            # [TP, 512]
    full = np.concatenate(blocks, axis=0).astype(np.float32)              # [F*NP, 2C]
    return full[:, :, None, None, None]


def kernel(
    person_feature,
    obj_feature,
    Wp,
    bp,
    Wpr,
    bpr,
    Wo,
    bo,
    Wm_obj,
    bm_obj,
    Wm_per,
    bm_per,
    f_num,
    np_pf,
    no_pf,
):
    assert int(f_num) == F and int(np_pf) == NP and int(no_pf) == NO
    pf = np.asarray(person_feature, dtype=np.float32)[:, :, 0, 0, 0]
    of = np.asarray(obj_feature, dtype=np.float32)[:, :, 0, 0, 0]
    args = [
        np.asarray(a, dtype=np.float32)
        for a in (Wp, bp, Wpr, bpr, Wo, bo, Wm_obj, bm_obj, Wm_per, bm_per)
    ]
    in_maps = _marshal(pf, of, *args)
    nc = _get_nc()
    res = run_bass_kernel_spmd(nc, in_maps, core_ids=list(range(NCORES)))
    return _unmarshal(res.results)


if __name__ == "__main__":
    # smoke test with random data against a numpy re-derivation
    rng = np.random.default_rng(0)
    pf = rng.standard_normal((F * NP, D, 1, 1, 1), dtype=np.float32)
    of = rng.standard_normal((F * NO, D, 1, 1, 1), dtype=np.float32)
    mk = lambda *s: (rng.standard_normal(s, dtype=np.float32) * 0.01)
    inputs = dict(
        person_feature=pf,
        obj_feature=of,
        Wp=mk(D, C),
        bp=np.zeros(C, np.float32),
        Wpr=mk(D, C),
        bpr=np.zeros(C, np.float32),
        Wo=mk(D, C),
        bo=np.zeros(C, np.float32),
        Wm_obj=rng.standard_normal((2 * C, C), dtype=np.float32) / np.sqrt(2 * C),
        bm_obj=np.zeros(C, np.float32),
        Wm_per=rng.standard_normal((2 * C, C), dtype=np.float32) / np.sqrt(2 * C),
        bm_per=np.zeros(C, np.float32),
        f_num=F,
        np_pf=NP,
        no_pf=NO,
    )
    out = kernel(**inputs)
    print("kernel output shape:", out.shape)
